# revision 6
# baseline (speedup 1.0000x reference)
"""Trainium2 Bass kernel v2 for the AxisMDTA dense-transformer block.

x (4, 256, 64, 256) fp32 -> out (4, 256, 64, 256) fp32.
Data-parallel over the 256 (b,t) samples across 8 NeuronCores (32/core).
Channel-major on-chip layout (c on partitions, (sample, f) on free dim).

v2 vs baseline:
- LN gammas folded into following weights host-side; LN bias + mean
  subtraction folded into the qkv / W1 matmuls as rank-k correction
  matmuls (shifted guarded rows handle the depthwise-conv taps exactly).
- Stats matmuls run f32r directly on fp32 x (no bf16 staging copy);
  squares via scalar_tensor_tensor (DVE 2x modes); dense Newton chains
  on gpsimd.
- Uniform [128,1024] PSUM quanta from one 4-deep ring.
- Stage emission is a skew-3 software-pipeline wavefront across the 4
  blocks so the PE stream always has ready work behind a stalled op.
- Guard columns zeroed once; per-block memsets eliminated.
- bf16 trunk (o1); x reloaded from DRAM for the proj residual; band
  moves via SWDGE (gpsimd) to offload HWDGE.
"""

import contextlib

import numpy as np
import ml_dtypes

import concourse.bass as bass
import concourse.mybir as mybir
import concourse.tile as tile
from concourse.vector_clock import ScopedClock
from concourse.bass_utils import run_bass_kernel_spmd

AF = mybir.ActivationFunctionType
ALU = mybir.AluOpType
DT = mybir.dt
BF16 = ml_dtypes.bfloat16

B, C, T, Fd = 4, 256, 64, 256
H, Dh = 8, 32
HID = 512
NCORES = 8
SPC = (B * T) // NCORES          # 32 samples per core
S = 8                            # samples per block
NBLK = SPC // S                  # 4 blocks
W = S * Fd                       # 2048 free columns per block
HW_ = W // 2                     # 1024-wide psum half
P3 = 260                         # corr row pitch (guarded)
PX = 258                         # xn guarded pitch
SKEW = 3
LN_EPS = 1e-5
RSQRT_MAGIC = 0x5F3759DF


class _TileContext(tile.TileContext):
    """Walrus in this container caps sync-wait commands per CTRL-class
    instruction; spread the exit drain's waits across single-wait nops."""

    def _drain_and_barrier(self, tick_clock, wait_clock):
        drain_inst = self.nc.sync.drain()
        wait_clock.add_sem_waits(
            drain_inst.ins, ScopedClock({None: tick_clock.global_clock})
        )
        si = drain_inst.ins.sync_info
        waits = list(si.on_wait or []) if si else []
        if len(waits) > 1:
            si.on_wait = waits[:1]
            for w in waits[1:]:
                n = self.nc.sync.nop(nofuse=True).ins
                n.sync_info = mybir.SyncInfo(on_wait=[w], on_update=[])
        self.nc.all_engine_barrier()
        assert self.sems is not None
        popped = self.nc._tile_sem_poison_stack.pop()
        assert popped is self._sem_poison
        self.nc.clear_and_free_semaphores(list(self.sems.allocated().values()))
        self.nc.all_engine_barrier()


def _f32r(ap):
    return ap.bitcast(DT.float32r)


def _brep(ap, nrep):
    """Insert a stride-0 replication dim after the partition dim."""
    ap.ap.insert(1, [0, nrep])
    return ap


def _split_excess_waits(nc, max_waits=2):
    """Walrus in this container caps sync-wait commands per instruction.
    Move excess waits onto same-engine NoOps inserted just before."""
    for f in nc.m.functions:
        for bb in f.blocks:
            new_insts = []
            for inst in bb.instructions:
                si = inst.sync_info
                waits = list(si.on_wait) if si and si.on_wait else []
                if len(waits) > max_waits:
                    si.on_wait = waits[:max_waits]
                    rest = waits[max_waits:]
                    for i in range(0, len(rest), max_waits):
                        nop = mybir.InstEventSemaphore(
                            name=f"I-ws{nc.next_id()}", ins=[], outs=[])
                        nop.engine = inst.engine
                        nop.sync_info = mybir.SyncInfo(
                            on_wait=rest[i:i + max_waits], on_update=[])
                        nc.register_instruction(nop)
                        new_insts.append(nop)
                new_insts.append(inst)
            bb.instructions[:] = new_insts


def build_nc():
    nc = bass.Bass()

    # ---- DRAM I/O ----
    x_in = nc.dram_tensor("x", [C, SPC, Fd], DT.float32, kind="ExternalInput")
    out_d = nc.dram_tensor("out", [C, SPC, Fd], DT.float32,
                           kind="ExternalOutput")
    wqkv2_d = nc.dram_tensor("wqkv2", [128, 2, 3 * C], DT.bfloat16,
                             kind="ExternalInput")
    corr2_d = nc.dram_tensor("corr2", [2, 3 * C], DT.bfloat16,
                             kind="ExternalInput")
    corrR_d = nc.dram_tensor("corrR", [2, 8 * 256], DT.bfloat16,
                             kind="ExternalInput")
    tapw_d = nc.dram_tensor("tapw", [128, 6, 2], DT.float32,
                            kind="ExternalInput")
    wproj_d = nc.dram_tensor("wproj", [128, 2, C], DT.bfloat16,
                             kind="ExternalInput")
    w1_d = nc.dram_tensor("w1", [128, 2, HID], DT.bfloat16,
                          kind="ExternalInput")
    w1cs_d = nc.dram_tensor("w1cs", [1, HID], DT.bfloat16,
                            kind="ExternalInput")
    w2_d = nc.dram_tensor("w2", [128, 4, C], DT.bfloat16,
                          kind="ExternalInput")
    b1v_d = nc.dram_tensor("b1v", [128, 4], DT.float32, kind="ExternalInput")
    b2v_d = nc.dram_tensor("b2v", [128, 2], DT.float32, kind="ExternalInput")
    temp_d = nc.dram_tensor("temp128", [128, 1], DT.float32,
                            kind="ExternalInput")
    onesb_d = nc.dram_tensor("onesb", [128, 128], DT.bfloat16,
                             kind="ExternalInput")
    bandh0_d = nc.dram_tensor("bandh0", [128, 128], DT.bfloat16,
                              kind="ExternalInput")
    bandh1_d = nc.dram_tensor("bandh1", [128, 128], DT.bfloat16,
                              kind="ExternalInput")
    rowind4_d = nc.dram_tensor("rowind4", [4, 128], DT.bfloat16,
                               kind="ExternalInput")
    ident_d = nc.dram_tensor("identb", [128, 128], DT.bfloat16,
                             kind="ExternalInput")


    with _TileContext(nc) as tc, contextlib.ExitStack() as ctx:
        cpool = ctx.enter_context(tc.tile_pool(name="consts", bufs=1))
        xpool = ctx.enter_context(tc.tile_pool(name="xp", bufs=2))
        xrpool = ctx.enter_context(tc.tile_pool(name="xrp", bufs=2))
        sqp = ctx.enter_context(tc.tile_pool(name="sqp", bufs=2))
        qkp = ctx.enter_context(tc.tile_pool(name="qkp", bufs=6))
        vvp = ctx.enter_context(tc.tile_pool(name="vvp", bufs=2))
        vpp = ctx.enter_context(tc.tile_pool(name="vpp", bufs=2))
        phap = ctx.enter_context(tc.tile_pool(name="phap", bufs=3))
        ostp = ctx.enter_context(tc.tile_pool(name="ostp", bufs=3))
        ohp = ctx.enter_context(tc.tile_pool(name="ohp", bufs=2))
        o1p = ctx.enter_context(tc.tile_pool(name="o1p", bufs=2))
        xhp = ctx.enter_context(tc.tile_pool(name="xhp", bufs=2))
        gelp = ctx.enter_context(tc.tile_pool(name="gelp", bufs=4))
        o2p = ctx.enter_context(tc.tile_pool(name="o2p", bufs=2))
        rows = ctx.enter_context(tc.tile_pool(name="rows", bufs=4))
        dense = ctx.enter_context(tc.tile_pool(name="dense", bufs=1))
        ps = ctx.enter_context(tc.tile_pool(name="ps", bufs=4, space="PSUM"))

        # ---- constants ----
        def cload(name, shape, dt, dram):
            t = cpool.tile(shape, dt, tag=name, name=name)
            nc.sync.dma_start(t[:], dram[:])
            return t

        wqkv_sb = cload("wqkv", [128, 2, 3 * C], DT.bfloat16, wqkv2_d)
        corr2_sb = cload("corr2", [2, 3 * C], DT.bfloat16, corr2_d)
        tapw = cload("tapw", [128, 6, 2], DT.float32, tapw_d)
        wproj_sb = cload("wproj", [128, 2, C], DT.bfloat16, wproj_d)
        w1_sb = cload("w1", [128, 2, HID], DT.bfloat16, w1_d)
        w1cs_sb = cload("w1cs", [1, HID], DT.bfloat16, w1cs_d)
        w2_sb = cload("w2", [128, 4, C], DT.bfloat16, w2_d)
        b1v = cload("b1v", [128, 4], DT.float32, b1v_d)
        b2v = cload("b2v", [128, 2], DT.float32, b2v_d)
        temp128 = cload("temp", [128, 1], DT.float32, temp_d)
        onesb = cload("onesb", [128, 128], DT.bfloat16, onesb_d)
        bandh = [cload("bandh0", [128, 128], DT.bfloat16, bandh0_d),
                 cload("bandh1", [128, 128], DT.bfloat16, bandh1_d)]
        rowind4 = cload("rowind4", [4, 128], DT.bfloat16, rowind4_d)
        identb = cload("identb", [128, 128], DT.bfloat16, ident_d)

        # xn: static pair, plain layout (conv guards live in qpre now)
        xn_st = [[cpool.tile([128, W], DT.bfloat16,
                             name=f"xn{i}_{kt}", tag=f"xn{i}_{kt}")
                  for kt in range(2)] for i in range(1)]
        # qpre: guarded staging for the depthwise conv (zero guard cols once)
        qpre_st = [cpool.tile([128, S, PX], DT.bfloat16,
                              name=f"qpre{i}", tag=f"qpre{i}")
                   for i in range(3)]
        for i in range(3):
            nc.vector.memset(qpre_st[i][:, :, 0:1], 0.0)
            nc.vector.memset(qpre_st[i][:, :, PX - 1:PX], 0.0)

        # ---- dense helpers (gpsimd newton chains, [*,128] tiles) ----
        def rsqrt_dense(x_ap, out_ap, iters=2, eng=None):
            eng = eng or nc.gpsimd
            shape = list(x_ap.shape)
            s1 = dense.tile(shape, DT.int32, tag="nw_i1", name="nw_i1")
            nc.vector.tensor_scalar(s1[:], x_ap.bitcast(DT.int32), 1, None,
                                    ALU.arith_shift_right)
            nc.vector.tensor_scalar(s1[:], s1[:], -1, None, ALU.bitwise_xor)
            nc.vector.tensor_scalar(s1[:], s1[:], RSQRT_MAGIC + 1, None,
                                    ALU.add)
            y = s1[:].bitcast(DT.float32)
            for it in range(iters):
                t = dense.tile(shape, DT.float32, tag="nw_t", name="nw_t")
                u = dense.tile(shape, DT.float32, tag="nw_u", name="nw_u")
                eng.tensor_mul(t[:], y, y)
                eng.tensor_scalar(t[:], t[:], -0.5, None, ALU.mult)
                eng.tensor_mul(u[:], t[:], x_ap)
                eng.tensor_scalar(u[:], u[:], 1.5, None, ALU.add)
                last = (it == iters - 1)
                ynew = out_ap if last else dense.tile(
                    shape, DT.float32, name="nw_y", tag="nw_y", bufs=2)
                yap = ynew if last else ynew[:]
                eng.tensor_mul(yap, u[:], y)
                y = yap

        def recip_dense(x_ap, out_ap, iters=2, eng=None):
            eng = eng or nc.gpsimd
            shape = list(x_ap.shape)
            s1 = dense.tile(shape, DT.int32, tag="nw_i1", name="nw_i1")
            nc.vector.tensor_scalar(s1[:], x_ap.bitcast(DT.int32), -1, None,
                                    ALU.bitwise_xor)
            nc.vector.tensor_scalar(s1[:], s1[:], 0x7EF127EA + 1, None,
                                    ALU.add)
            y = s1[:].bitcast(DT.float32)
            for it in range(iters):
                u = dense.tile(shape, DT.float32, tag="nw_t", name="nw_t")
                eng.tensor_mul(u[:], x_ap, y)
                v = dense.tile(shape, DT.float32, tag="nw_u", name="nw_u")
                eng.tensor_scalar(v[:], u[:], -1.0, None, ALU.mult)
                eng.tensor_scalar(v[:], v[:], 2.0, None, ALU.add)
                last = (it == iters - 1)
                ynew = out_ap if last else dense.tile(
                    shape, DT.float32, name="nw_y", tag="nw_y", bufs=2)
                yap = ynew if last else ynew[:]
                eng.tensor_mul(yap, v[:], y)
                y = yap

        st = {b: {} for b in range(NBLK)}

        # ---------------- stages ----------------
        def s0_load(blk):
            d = st[blk]
            d["xb"] = [sqp.tile([128, W], DT.bfloat16, name=f"xb_{kt}",
                                tag="xb") for kt in range(2)]
            for kt in range(2):
                for h2 in range(2):
                    xh32 = xpool.tile([128, S // 2, Fd], DT.float32,
                                      name="xh32", tag="x32")
                    nc.sync.dma_start(
                        xh32[:],
                        x_in[kt * 128:(kt + 1) * 128,
                             blk * S + h2 * 4:blk * S + (h2 + 1) * 4, :])
                    nc.vector.tensor_copy(
                        d["xb"][kt][:, h2 * HW_:(h2 + 1) * HW_],
                        xh32[:].rearrange("p s f -> p (s f)"))

        def _stats(blk, srcs, sqs, which, fr):
            """Partition-sum stats of srcs (and sqs) -> dense mu*rsig and
            rsig rows (bf16 [16,128]) stored as mr{which}/rr{which}."""
            d = st[blk]
            su_row = rows.tile([1, W], DT.bfloat16,
                               name=f"su_{which}", tag="rows")
            sq_row = rows.tile([1, W], DT.bfloat16,
                               name=f"sqr_{which}", tag="rows")
            for h2 in range(2):
                ps_su = ps.tile([1, HW_], DT.float32, tag="mm", name="ps_su")
                ps_sq = ps.tile([1, HW_], DT.float32, tag="mm", name="ps_sq")
                for ch in range(2):
                    sl = slice(h2 * HW_ + ch * 512, h2 * HW_ + (ch + 1) * 512)
                    psl = slice(ch * 512, (ch + 1) * 512)
                    for kt in range(2):
                        nc.tensor.matmul(
                            ps_su[0:1, psl], onesb[:, 0:1],
                            srcs[kt][:, sl],
                            start=(kt == 0), stop=(kt == 1),
                            skip_group_check=True)
                    for kt in range(2):
                        nc.tensor.matmul(
                            ps_sq[0:1, psl], onesb[:, 0:1],
                            sqs[(kt, h2)][:, psl],
                            start=(kt == 0), stop=(kt == 1),
                            skip_group_check=True)
                nc.scalar.activation(su_row[0:1, h2 * HW_:(h2 + 1) * HW_],
                                     ps_su[:], AF.Copy)
                nc.scalar.activation(sq_row[0:1, h2 * HW_:(h2 + 1) * HW_],
                                     ps_sq[:], AF.Copy)
            dsu = dense.tile([16, 128], DT.bfloat16, tag="dsu", name="dsu")
            dsq = dense.tile([16, 128], DT.bfloat16, tag="dsq", name="dsq")
            nc.sync.dma_start(
                dsu[:], su_row[:].rearrange("o (j c) -> o j c", c=128))
            nc.sync.dma_start(
                dsq[:], sq_row[:].rearrange("o (j c) -> o j c", c=128))
            mu = dense.tile([16, 128], DT.float32, tag="dmu", name="dmu")
            nc.vector.tensor_scalar(mu[:], dsu[:], 1.0 / C, None, ALU.mult)
            var = dense.tile([16, 128], DT.float32, tag="dvar", name="dvar")
            m2 = dense.tile([16, 128], DT.float32, tag="nw_t", name="dm2")
            nc.vector.tensor_mul(m2[:], mu[:], mu[:])
            nc.vector.tensor_scalar(var[:], dsq[:], 1.0 / C, LN_EPS,
                                    ALU.mult, ALU.add)
            nc.vector.tensor_sub(var[:], var[:], m2[:])
            rsd = dense.tile([16, 128], DT.float32, tag="drs", name="drs")
            rsqrt_dense(var[:], rsd[:], iters=1, eng=nc.vector)
            mr = dense.tile([16, 128], DT.bfloat16, tag="dmr", name="dmr")
            nc.vector.tensor_mul(mr[:], mu[:], rsd[:])
            rr = dense.tile([16, 128], DT.bfloat16, tag="drr", name="drr")
            nc.vector.tensor_copy(rr[:], rsd[:])
            d[f"mr{which}"] = mr
            d[f"rr{which}"] = rr

        def s1_ln1_stats(blk):
            d = st[blk]
            srcs = [d["xb"][kt][:] for kt in range(2)]
            sqs = {}
            for kt in range(2):
                for h2 in range(2):
                    x2 = sqp.tile([128, HW_], DT.bfloat16, name="x2",
                                  tag="sqh")
                    s_ = srcs[kt][:, h2 * HW_:(h2 + 1) * HW_]
                    nc.vector.scalar_tensor_tensor(
                        x2[:], s_, 1.0, s_, ALU.mult, ALU.mult)
                    sqs[(kt, h2)] = x2
            _stats(blk, srcs, sqs, 1, fr=False)

        def s2_ln1_apply(blk):
            d = st[blk]
            mr, rr = d.pop("mr1"), d.pop("rr1")
            rrow = rows.tile([1, W], DT.bfloat16, name="rrow", tag="rows")
            nc.sync.dma_start(
                rrow[:].rearrange("o (j c) -> o j c", c=128), rr[:])
            corrR = rows.tile([2, W], DT.bfloat16, name="corrR", tag="rows")
            nc.sync.dma_start(
                corrR[0:1, :].rearrange("o (j c) -> o j c", c=128), mr[:])
            nc.sync.dma_start(corrR[1:2, :], corrR_d[1:2, :])
            d["corrR"] = corrR
            xn16 = xn_st[0]
            for h2 in range(2):
                ps_r = ps.tile([128, HW_], DT.float32, tag="mm", name="ps_r")
                for ch in range(2):
                    nc.tensor.matmul(
                        ps_r[:, ch * 512:(ch + 1) * 512],
                        onesb[0:1, :],
                        rrow[0:1, h2 * HW_ + ch * 512:
                             h2 * HW_ + (ch + 1) * 512],
                        start=True, stop=True, skip_group_check=True)
                for kt in range(2):
                    nc.vector.tensor_mul(
                        xn16[kt][:, h2 * HW_:(h2 + 1) * HW_],
                        d["xb"][kt][:, h2 * HW_:(h2 + 1) * HW_],
                        ps_r[:])
            d["xn"] = xn16

        def qkv_m(blk, m, qc):
            d = st[blk]
            qp3 = qpre_st[m % 3][:]
            if m < 4:
                qt = qkp.tile([128, W], DT.bfloat16, name=f"qc{m}", tag="qk")
            else:
                qt = vvp.tile([128, W], DT.bfloat16, name=f"vc{m}", tag="vv")
            qc.append(qt)
            for h2 in range(2):
                ps_m = ps.tile([128, HW_], DT.float32, tag="mm", name="ps_m")
                for ch in range(2):
                    sl = slice(h2 * HW_ + ch * 512, h2 * HW_ + (ch + 1) * 512)
                    for kt in range(2):
                        nc.tensor.matmul(
                            ps_m[:, ch * 512:(ch + 1) * 512],
                            wqkv_sb[:, kt, m * 128:(m + 1) * 128],
                            d["xn"][kt][:, sl],
                            start=(kt == 0), stop=False,
                            skip_group_check=True)
                    nc.tensor.matmul(
                        ps_m[:, ch * 512:(ch + 1) * 512],
                        corr2_sb[:, m * 128:(m + 1) * 128],
                        d["corrR"][:, sl],
                        start=False, stop=True, skip_group_check=True)
                nc.scalar.activation(
                    qp3[:, h2 * 4:(h2 + 1) * 4, 1:1 + Fd],
                    ps_m[:].rearrange("p (s f) -> p s f", f=Fd),
                    AF.Copy)
            # depthwise conv3 along f (middle tap folded into Wqkv):
            # qc = qpre + r0*shift(-1) + r2*shift(+1), in place (bf16 DVE)
            vm1 = qp3[:, :, 0:Fd]
            v00 = qp3[:, :, 1:1 + Fd]
            vp1 = qp3[:, :, 2:2 + Fd]
            qf = qc[m][:].rearrange("p (s f) -> p s f", f=Fd)
            nc.vector.scalar_tensor_tensor(qf, vm1, tapw[:, m, 0:1], v00,
                                           ALU.mult, ALU.add)
            nc.vector.scalar_tensor_tensor(qf, vp1, tapw[:, m, 1:2], qf,
                                           ALU.mult, ALU.add)

        def qkv_tail(blk, qc):
            d = st[blk]
            d.pop("corrR")
            d["qc"] = qc[:4]
            # v transpose (frees v tiles fast)
            vp = [vpp.tile([128, S, H, Dh + 1], DT.bfloat16,
                           name=f"vp{b2}", tag=f"vp{b2}") for b2 in range(2)]
            for b2 in range(2):
                nc.vector.memset(vp[b2][:, :, :, Dh:Dh + 1], 1.0)
            for ti in range(2):
                vt = qc[4 + ti]
                for b2 in range(2):
                    ps_tp = ps.tile([128, S * 128], DT.bfloat16, tag="mm",
                                    name="ps_tp")
                    for s in range(S):
                        nc.tensor.transpose(
                            ps_tp[:, s * 128:(s + 1) * 128],
                            vt[:, s * Fd + b2 * 128:s * Fd + b2 * 128 + 128],
                            identb[:])
                    nc.vector.tensor_copy(
                        vp[b2][:, :, 4 * ti:4 * ti + 4, 0:Dh],
                        ps_tp[:].rearrange("p (s hb d) -> p s hb d",
                                           s=S, hb=4))
            d["vp"] = vp

        def s3_qkv(blk):
            qc = []
            for m in range(6):
                qkv_m(blk, m, qc)
            qkv_tail(blk, qc)

        def s4_l2sumsq(blk):
            d = st[blk]
            d["dnq"] = {}
            for vi, base in (("q", 0), ("k", 2)):
                d_n = dense.tile([128, 128], DT.bfloat16, tag="dn", name="dn")
                for h2 in range(2):
                    ps_n = ps.tile([128, HW_], DT.float32, tag="mm",
                                   name="ps_n")
                    for ti in range(2):
                        sq = sqp.tile([128, HW_], DT.bfloat16, name="l2sq",
                                      tag="sqh")
                        qs = d["qc"][base + ti][:, h2 * HW_:(h2 + 1) * HW_]
                        nc.vector.scalar_tensor_tensor(
                            sq[:], qs, 1.0, qs, ALU.mult, ALU.mult)
                        for ch in range(2):
                            nc.tensor.matmul(
                                ps_n[:, ch * 512:(ch + 1) * 512],
                                bandh[ti][:], sq[:, ch * 512:(ch + 1) * 512],
                                start=(ti == 0), stop=(ti == 1),
                                skip_group_check=True)
                    nsb = rows.tile([8, HW_], DT.bfloat16, name="nsb",
                                    tag="rows")
                    nc.any.tensor_copy(nsb[:], ps_n[0:8, :])
                    nc.sync.dma_start(
                        d_n[h2 * 64:(h2 + 1) * 64, :],
                        nsb[:].rearrange("h (j c) -> h j c", c=128))
                dnf = dense.tile([128, 128], DT.float32, tag="dnf",
                                 name="dnf")
                nc.vector.tensor_copy(dnf[:], d_n[:])
                r_n = dense.tile([128, 128], DT.float32, tag="dr", name="dr")
                rsqrt_dense(dnf[:], r_n[:], iters=1, eng=nc.vector)
                r16 = dense.tile([128, 128], DT.bfloat16, tag="dr16",
                                 name="dr16")
                if vi == "k":
                    nc.vector.tensor_scalar(r16[:], r_n[:], temp128[:, 0:1],
                                            None, ALU.mult)
                else:
                    nc.vector.tensor_copy(r16[:], r_n[:])
                d["dnq"][vi] = r16

        def s5_l2apply(blk):
            d = st[blk]
            for vi, base in (("q", 0), ("k", 2)):
                r16 = d["dnq"].pop(vi)
                for ti in range(2):
                    r4 = rows.tile([4, W], DT.bfloat16, name="r4", tag="rows")
                    for h2 in range(2):
                        nc.sync.dma_start(
                            r4[:, h2 * HW_:(h2 + 1) * HW_].rearrange(
                                "b (j c) -> b j c", c=128),
                            r16[h2 * 64 + ti * 32:h2 * 64 + ti * 32 + 32, :])
                    for h2 in range(2):
                        ps_b = ps.tile([128, HW_], DT.float32, tag="mm",
                                       name="ps_b")
                        for ch in range(2):
                            sl = slice(h2 * HW_ + ch * 512,
                                       h2 * HW_ + (ch + 1) * 512)
                            nc.tensor.matmul(
                                ps_b[:, ch * 512:(ch + 1) * 512],
                                rowind4[:], r4[:, sl],
                                start=True, stop=True, skip_group_check=True)
                        qs = d["qc"][base + ti][:, h2 * HW_:(h2 + 1) * HW_]
                        nc.vector.tensor_mul(qs, qs, ps_b[:])
            d.pop("dnq")

        def attn_h(blk, h, oh16, d_rs):
            d = st[blk]
            ti, band = h // 4, (h % 4) * 32
            phat = []
            for jt in range(2):
                pj = phap.tile([128, W], DT.bfloat16, tag="phat",
                               name="phat")
                for h2 in range(2):
                    ps_S = ps.tile([128, HW_], DT.float32, tag="mm",
                                   name="ps_S")
                    for si in range(4):
                        s = h2 * 4 + si
                        nc.tensor.matmul(
                            ps_S[:, si * Fd:(si + 1) * Fd],
                            d["qc"][2 + ti][
                                band:band + 32,
                                s * Fd + jt * 128:s * Fd + jt * 128 + 128],
                            d["qc"][ti][band:band + 32,
                                        s * Fd:(s + 1) * Fd],
                            start=True, stop=True, skip_group_check=True,
                            tile_position=(band, 0))
                    nc.scalar.activation(
                        pj[:, h2 * HW_:(h2 + 1) * HW_], ps_S[:], AF.Exp)
                phat.append(pj)
            ostg = ostp.tile([Dh + 1, W], DT.bfloat16, tag="ostg",
                             name="ostg")
            for h2 in range(2):
                ps_O = ps.tile([Dh + 1, HW_], DT.float32, tag="mm",
                               name="ps_O")
                for si in range(4):
                    s = h2 * 4 + si
                    for ktj in range(2):
                        nc.tensor.matmul(
                            ps_O[:, si * Fd:(si + 1) * Fd],
                            d["vp"][ktj][:, s, h, :],
                            phat[ktj][:, s * Fd:(s + 1) * Fd],
                            start=(ktj == 0), stop=(ktj == 1),
                            skip_group_check=True)
                nc.any.tensor_copy(ostg[:, h2 * HW_:(h2 + 1) * HW_],
                                   ps_O[:])
            nc.gpsimd.dma_start(oh16[ti][band:band + 32, :],
                                ostg[0:Dh, :])
            nc.sync.dma_start(
                d_rs[h * 16:(h + 1) * 16, :],
                ostg[Dh:Dh + 1, :].rearrange("o (j c) -> o j c", c=128))

        def attn_head_setup(blk):
            d = st[blk]
            oh16 = [ohp.tile([128, W], DT.bfloat16, name=f"oh{ti}", tag="oh")
                    for ti in range(2)]
            d_rs = dense.tile([128, 128], DT.bfloat16, tag="dnr", name="dnr")
            d["oh16"] = oh16
            return oh16, d_rs

        def attn_tail(blk, d_rs):
            d = st[blk]
            drf = dense.tile([128, 128], DT.float32, tag="drf", name="drf")
            nc.vector.tensor_copy(drf[:], d_rs[:])
            d_ri = dense.tile([128, 128], DT.bfloat16, tag="dri", name="dri")
            recip_dense(drf[:], d_ri[:], eng=nc.vector)
            d["d_ri"] = d_ri

        def s6_attn(blk):
            oh16, d_rs = attn_head_setup(blk)
            for h in range(H):
                attn_h(blk, h, oh16, d_rs)
            attn_tail(blk, d_rs)

        def merged_attn_qkv(ba, bq):
            """Interleave attention(ba) heads with qkv(bq) m-tiles so the
            shared psum ring rotates through both stages."""
            oh16, d_rs = attn_head_setup(ba)
            qc = []
            plan = ["h0", "h1", "m0", "h2", "m1", "h3", "m2", "h4", "m3",
                    "h5", "m4", "h6", "m5", "h7"]
            for step in plan:
                if step[0] == "h":
                    attn_h(ba, int(step[1]), oh16, d_rs)
                else:
                    qkv_m(bq, int(step[1]), qc)
            qkv_tail(bq, qc)
            attn_tail(ba, d_rs)

        def s7_proj(blk):
            d = st[blk]
            d_ri = d.pop("d_ri")
            for ti in range(2):
                r4 = rows.tile([4, W], DT.bfloat16, name="rinv", tag="rows")
                nc.sync.dma_start(
                    r4[:].rearrange("b (j c) -> b j c", c=128),
                    d_ri[ti * 64:(ti + 1) * 64, :])
                for h2 in range(2):
                    ps_b = ps.tile([128, HW_], DT.float32, tag="mm",
                                   name="ps_b2")
                    for ch in range(2):
                        sl = slice(h2 * HW_ + ch * 512,
                                   h2 * HW_ + (ch + 1) * 512)
                        nc.tensor.matmul(
                            ps_b[:, ch * 512:(ch + 1) * 512],
                            rowind4[:], r4[:, sl],
                            start=True, stop=True, skip_group_check=True)
                    ohs = d["oh16"][ti][:, h2 * HW_:(h2 + 1) * HW_]
                    nc.vector.tensor_mul(ohs, ohs, ps_b[:])
            o1 = [o1p.tile([128, W], DT.bfloat16, name=f"o1_{m}", tag="o1")
                  for m in range(2)]
            for m2 in range(2):
                for h2 in range(2):
                    xr32 = xrpool.tile([128, S // 2, Fd], DT.float32,
                                       name="xr32", tag="xr32")
                    nc.sync.dma_start(
                        xr32[:],
                        x_in[m2 * 128:(m2 + 1) * 128,
                             blk * S + h2 * 4:blk * S + (h2 + 1) * 4, :])
                    xf = xr32[:].rearrange("p s f -> p (s f)")
                    ps_y = ps.tile([128, HW_], DT.float32, tag="mm",
                                   name="ps_y")
                    for ch in range(2):
                        sl = slice(h2 * HW_ + ch * 512,
                                   h2 * HW_ + (ch + 1) * 512)
                        for kt in range(2):
                            nc.tensor.matmul(
                                ps_y[:, ch * 512:(ch + 1) * 512],
                                wproj_sb[:, kt, m2 * 128:(m2 + 1) * 128],
                                d["oh16"][kt][:, sl],
                                start=(kt == 0), stop=(kt == 1),
                                skip_group_check=True)
                    nc.vector.scalar_tensor_tensor(
                        o1[m2][:, h2 * HW_:(h2 + 1) * HW_],
                        ps_y[:], 1.0, xf[:], ALU.mult, ALU.add)
            d["o1"] = o1
            d.pop("oh16")
            d.pop("qc")
            d.pop("vp")
            d.pop("xn")
            d.pop("xb")

        def s8_ln2_stats(blk):
            d = st[blk]
            srcs = [d["o1"][kt][:] for kt in range(2)]
            sqs = {}
            for kt in range(2):
                for h2 in range(2):
                    x2 = sqp.tile([128, HW_], DT.bfloat16, name="o1sq",
                                  tag="sqh")
                    s_ = srcs[kt][:, h2 * HW_:(h2 + 1) * HW_]
                    nc.vector.scalar_tensor_tensor(
                        x2[:], s_, 1.0, s_, ALU.mult, ALU.mult)
                    sqs[(kt, h2)] = x2
            _stats(blk, srcs, sqs, 2, fr=False)

        def mlp_head(blk):
            d = st[blk]
            mr, rr = d.pop("mr2"), d.pop("rr2")
            rrow = rows.tile([1, W], DT.bfloat16, name="rrow2", tag="rows")
            nc.sync.dma_start(
                rrow[:].rearrange("o (j c) -> o j c", c=128), rr[:])
            mrow = rows.tile([1, W], DT.bfloat16, name="mrow2", tag="rows")
            nc.sync.dma_start(
                mrow[:].rearrange("o (j c) -> o j c", c=128), mr[:])
            xh = [xhp.tile([128, W], DT.bfloat16, name=f"xh{kt}", tag="xh")
                  for kt in range(2)]
            for h2 in range(2):
                ps_r = ps.tile([128, HW_], DT.float32, tag="mm", name="ps_r2")
                for ch in range(2):
                    nc.tensor.matmul(
                        ps_r[:, ch * 512:(ch + 1) * 512],
                        onesb[0:1, :],
                        rrow[0:1, h2 * HW_ + ch * 512:
                             h2 * HW_ + (ch + 1) * 512],
                        start=True, stop=True, skip_group_check=True)
                for kt in range(2):
                    nc.vector.tensor_mul(
                        xh[kt][:, h2 * HW_:(h2 + 1) * HW_],
                        d["o1"][kt][:, h2 * HW_:(h2 + 1) * HW_],
                        ps_r[:])
            g16 = [gelp.tile([128, W], DT.bfloat16, name=f"gel{m}", tag="gel")
                   for m in range(4)]
            d["xh"] = xh
            d["mrow"] = mrow
            d["g16"] = g16

        def mlp_w1(blk, mh):
            d = st[blk]
            xh, mrow, g16 = d["xh"], d["mrow"], d["g16"]
            for h2 in range(2):
                ps_h = ps.tile([128, HW_], DT.float32, tag="mm",
                               name="ps_h")
                for ch in range(2):
                    sl = slice(h2 * HW_ + ch * 512,
                               h2 * HW_ + (ch + 1) * 512)
                    for kt in range(2):
                        nc.tensor.matmul(
                            ps_h[:, ch * 512:(ch + 1) * 512],
                            w1_sb[:, kt, mh * 128:(mh + 1) * 128],
                            xh[kt][:, sl],
                            start=(kt == 0), stop=False,
                            skip_group_check=True)
                    nc.tensor.matmul(
                        ps_h[:, ch * 512:(ch + 1) * 512],
                        w1cs_sb[0:1, mh * 128:(mh + 1) * 128],
                        mrow[0:1, sl],
                        start=False, stop=True, skip_group_check=True)
                nc.scalar.activation(
                    g16[mh][:, h2 * HW_:(h2 + 1) * HW_], ps_h[:],
                    AF.Gelu, bias=b1v[:, mh:mh + 1], scale=1.0)

        def mlp_w2(blk, m2, h2):
            d = st[blk]
            g16 = d["g16"]
            ps_y = ps.tile([128, HW_], DT.float32, tag="mm", name="ps_y2")
            for ch in range(2):
                for kt in range(4):
                    nc.tensor.matmul(
                        ps_y[:, ch * 512:(ch + 1) * 512],
                        w2_sb[:, kt, m2 * 128:(m2 + 1) * 128],
                        g16[kt][:, h2 * HW_ + ch * 512:
                                h2 * HW_ + (ch + 1) * 512],
                        start=(kt == 0), stop=(kt == 3),
                        skip_group_check=True)
            o2 = o2p.tile([128, HW_], DT.float32, tag="o2", name="o2t")
            nc.vector.scalar_tensor_tensor(
                o2[:], ps_y[:], b2v[:, m2:m2 + 1],
                d["o1"][m2][:, h2 * HW_:(h2 + 1) * HW_],
                ALU.add, ALU.add)
            nc.sync.dma_start(
                out_d[m2 * 128:(m2 + 1) * 128,
                      blk * S + h2 * 4:blk * S + (h2 + 1) * 4, :],
                o2[:].rearrange("p (s f) -> p s f", f=Fd))

        def mlp_tail(blk):
            d = st[blk]
            d.pop("xh")
            d.pop("mrow")
            d.pop("g16")
            d.pop("o1")

        def s9_mlp(blk):
            mlp_head(blk)
            for mh in range(4):
                mlp_w1(blk, mh)
            for m2 in range(2):
                for h2 in range(2):
                    mlp_w2(blk, m2, h2)
            mlp_tail(blk)

        def merged_step(bm, ba, bq):
            """Interleave mlp(bm), attention(ba), qkv(bq); any may be None."""
            if ba is not None:
                oh16, d_rs = attn_head_setup(ba)
            if bm is not None:
                mlp_head(bm)
            qc = []
            plan = []
            for i in range(8):
                if ba is not None:
                    plan.append(("h", i))
                if bm is not None and i < 4:
                    plan.append(("w1", i))
                if bm is not None and 4 <= i < 8:
                    plan.append(("w2", i - 4))
                if bq is not None and 1 <= i < 7:
                    plan.append(("m", i - 1))
            for kind, i in plan:
                if kind == "h":
                    attn_h(ba, i, oh16, d_rs)
                elif kind == "w1":
                    mlp_w1(bm, i)
                elif kind == "w2":
                    mlp_w2(bm, i // 2, i % 2)
                else:
                    qkv_m(bq, i, qc)
            if bq is not None:
                qkv_tail(bq, qc)
            if ba is not None:
                attn_tail(ba, d_rs)
            if bm is not None:
                mlp_tail(bm)

        stages = [s0_load, s1_ln1_stats, s2_ln1_apply, s3_qkv, s4_l2sumsq,
                  s5_l2apply, s6_attn, s7_proj, s8_ln2_stats, s9_mlp]


        # skew-3 wavefront: later stages (lower block index) first.
        # s9(b-1), s6(b), s3(b+1) land on the same step; emit them
        # interleaved so the shared psum ring rotates through all three.
        nst = len(stages)
        for t in range(nst + SKEW * (NBLK - 1)):
            todo = [(b, t - SKEW * b) for b in range(NBLK)
                    if 0 <= t - SKEW * b < nst]
            jmap = {j: b for (b, j) in todo}
            skip = set()
            if 6 in jmap:
                bm = jmap.get(9)
                ba = jmap[6]
                bq = jmap.get(3)
                for j in (9, 6, 3):
                    if j in jmap:
                        skip.add((jmap[j], j))
            order = sorted(todo, key=lambda bj: (0 if bj[1] == 4 else 1,
                                                 bj[0]))
            for b, j in order:
                if (b, j) in skip:
                    if j == 6:
                        merged_step(jmap.get(9), b, jmap.get(3))
                    continue
                stages[j](b)

    _split_excess_waits(nc, max_waits=1)
    return nc


def _host_prep(inputs):
    Wqkv = np.asarray(inputs["Wqkv"], np.float64)        # (C, 3C)
    g1 = np.asarray(inputs["norm1_g"], np.float64)
    b1n = np.asarray(inputs["norm1_b"], np.float64)
    g2 = np.asarray(inputs["norm2_g"], np.float64)
    b2n = np.asarray(inputs["norm2_b"], np.float64)
    dw_w = np.asarray(inputs["dw_w"], np.float64)
    taps = dw_w[:, 0, :]                                 # (3C, 3)

    Wq = Wqkv * g1[:, None]                              # g1 folded
    # fold the middle conv tap into the weights; conv uses tap ratios
    w1t = taps[:, 1].copy()
    w1t = np.where(np.abs(w1t) < 1e-30, 1e-30, w1t)
    Wqf = Wq * w1t[None, :]
    wqkv2 = np.ascontiguousarray(
        Wqf.reshape(2, 128, 3 * C).transpose(1, 0, 2)).astype(BF16)

    colsum = Wqf.sum(axis=0)                             # (3C,)
    bq = (b1n @ Wqkv) * w1t                              # (3C,)
    corr2 = np.stack([-colsum, bq]).astype(BF16)         # (2, 3C)
    corrR = np.zeros((2, S * Fd), np.float32)
    corrR[1] = 1.0                                       # static ones row
    ratios = np.stack([taps[:, 0] / w1t, taps[:, 2] / w1t], axis=1)
    tapw = np.ascontiguousarray(
        ratios.reshape(6, 128, 2).transpose(1, 0, 2)).astype(np.float32)

    def kt_major(w, nkt):
        K, N = w.shape
        return np.ascontiguousarray(
            w.reshape(nkt, 128, N).transpose(1, 0, 2)).astype(BF16)

    wproj = kt_major(np.asarray(inputs["Wproj"], np.float64), 2)
    W1 = np.asarray(inputs["W1"], np.float64)
    W1g = W1 * g2[:, None]
    w1 = kt_major(W1g, 2)
    w1cs = (-W1g.sum(axis=0)).reshape(1, HID).astype(BF16)
    w2 = kt_major(np.asarray(inputs["W2"], np.float64), 4)

    b1p = np.asarray(inputs["b1"], np.float64) + b2n @ W1
    b1v = np.ascontiguousarray(b1p.reshape(4, 128).T).astype(np.float32)
    b2v = np.ascontiguousarray(
        np.asarray(inputs["b2"], np.float32).reshape(2, 128).T)

    temp = np.asarray(inputs["temperature"], np.float32).reshape(H)
    # l2 dense rows are h2-major: row = h2*64 + head*8 + j
    temp128 = np.array([temp[(r % 64) // 8] for r in range(128)],
                       np.float32).reshape(128, 1)

    bandh0 = np.zeros((128, 128), np.float32)
    bandh1 = np.zeros((128, 128), np.float32)
    for dd in range(128):
        for m in range(128):
            if m % 8 == dd // 32:
                bandh0[dd, m] = 1.0
            if m % 8 == 4 + dd // 32:
                bandh1[dd, m] = 1.0
    rowind4 = np.zeros((4, 128), np.float32)
    for m in range(128):
        rowind4[m // 32, m] = 1.0

    return dict(
        wqkv2=wqkv2, corr2=corr2, corrR=corrR.astype(BF16), tapw=tapw,
        wproj=wproj, w1=w1, w1cs=w1cs, w2=w2,
        b1v=b1v, b2v=b2v, temp128=temp128,
        onesb=np.ones((128, 128), BF16),
        bandh0=bandh0.astype(BF16),
        bandh1=bandh1.astype(BF16),
        rowind4=rowind4.astype(BF16),
        identb=np.eye(128).astype(BF16),
    )


_NC_CACHE = {}


def get_nc():
    if "nc" not in _NC_CACHE:
        _NC_CACHE["nc"] = build_nc()
    return _NC_CACHE["nc"]


def make_in_maps(inputs):
    consts = _host_prep(inputs)
    x = np.asarray(inputs["x"], np.float32)  # (B, C, T, Fd)
    in_maps = []
    for core in range(NCORES):
        b, t0 = core // 2, (core % 2) * SPC
        m = dict(consts)
        m["x"] = np.ascontiguousarray(x[b, :, t0:t0 + SPC, :])
        in_maps.append(m)
    return in_maps


def assemble_out(results):
    out = np.zeros((B, C, T, Fd), np.float32)
    for core in range(NCORES):
        b, t0 = core // 2, (core % 2) * SPC
        out[b, :, t0:t0 + SPC, :] = results[core]["out"]
    return out


def kernel(**inputs):
    nc = get_nc()
    in_maps = make_in_maps(inputs)
    res = run_bass_kernel_spmd(nc, in_maps, core_ids=list(range(NCORES)))
    return assemble_out(res.results)


# revision 7
# speedup vs baseline: 1.0204x; 1.0204x over previous
"""Trainium2 Bass kernel v2 for the AxisMDTA dense-transformer block.

x (4, 256, 64, 256) fp32 -> out (4, 256, 64, 256) fp32.
Data-parallel over the 256 (b,t) samples across 8 NeuronCores (32/core).
Channel-major on-chip layout (c on partitions, (sample, f) on free dim).

v2 vs baseline:
- LN gammas folded into following weights host-side; LN bias + mean
  subtraction folded into the qkv / W1 matmuls as rank-k correction
  matmuls (shifted guarded rows handle the depthwise-conv taps exactly).
- Stats matmuls run f32r directly on fp32 x (no bf16 staging copy);
  squares via scalar_tensor_tensor (DVE 2x modes); dense Newton chains
  on gpsimd.
- Uniform [128,1024] PSUM quanta from one 4-deep ring.
- Stage emission is a skew-3 software-pipeline wavefront across the 4
  blocks so the PE stream always has ready work behind a stalled op.
- Guard columns zeroed once; per-block memsets eliminated.
- bf16 trunk (o1); x reloaded from DRAM for the proj residual; band
  moves via SWDGE (gpsimd) to offload HWDGE.
"""

import contextlib

import numpy as np
import ml_dtypes

import concourse.bass as bass
import concourse.mybir as mybir
import concourse.tile as tile
from concourse.vector_clock import ScopedClock
from concourse.bass_utils import run_bass_kernel_spmd

AF = mybir.ActivationFunctionType
ALU = mybir.AluOpType
DT = mybir.dt
BF16 = ml_dtypes.bfloat16

B, C, T, Fd = 4, 256, 64, 256
H, Dh = 8, 32
HID = 512
NCORES = 8
SPC = (B * T) // NCORES          # 32 samples per core
S = 8                            # samples per block
NBLK = SPC // S                  # 4 blocks
W = S * Fd                       # 2048 free columns per block
HW_ = W // 2                     # 1024-wide psum half
P3 = 260                         # corr row pitch (guarded)
PX = 258                         # xn guarded pitch
SKEW = 3
LN_EPS = 1e-5
RSQRT_MAGIC = 0x5F3759DF


class _TileContext(tile.TileContext):
    """Walrus in this container caps sync-wait commands per CTRL-class
    instruction; spread the exit drain's waits across single-wait nops."""

    def _drain_and_barrier(self, tick_clock, wait_clock):
        drain_inst = self.nc.sync.drain()
        wait_clock.add_sem_waits(
            drain_inst.ins, ScopedClock({None: tick_clock.global_clock})
        )
        si = drain_inst.ins.sync_info
        waits = list(si.on_wait or []) if si else []
        if len(waits) > 1:
            si.on_wait = waits[:1]
            for w in waits[1:]:
                n = self.nc.sync.nop(nofuse=True).ins
                n.sync_info = mybir.SyncInfo(on_wait=[w], on_update=[])
        self.nc.all_engine_barrier()
        assert self.sems is not None
        popped = self.nc._tile_sem_poison_stack.pop()
        assert popped is self._sem_poison
        self.nc.clear_and_free_semaphores(list(self.sems.allocated().values()))
        self.nc.all_engine_barrier()


def _f32r(ap):
    return ap.bitcast(DT.float32r)


def _brep(ap, nrep):
    """Insert a stride-0 replication dim after the partition dim."""
    ap.ap.insert(1, [0, nrep])
    return ap


def _split_excess_waits(nc, max_waits=2):
    """Walrus in this container caps sync-wait commands per instruction.
    Move excess waits onto same-engine NoOps inserted just before."""
    for f in nc.m.functions:
        for bb in f.blocks:
            new_insts = []
            for inst in bb.instructions:
                si = inst.sync_info
                waits = list(si.on_wait) if si and si.on_wait else []
                if len(waits) > max_waits:
                    si.on_wait = waits[:max_waits]
                    rest = waits[max_waits:]
                    for i in range(0, len(rest), max_waits):
                        nop = mybir.InstEventSemaphore(
                            name=f"I-ws{nc.next_id()}", ins=[], outs=[])
                        nop.engine = inst.engine
                        nop.sync_info = mybir.SyncInfo(
                            on_wait=rest[i:i + max_waits], on_update=[])
                        nc.register_instruction(nop)
                        new_insts.append(nop)
                new_insts.append(inst)
            bb.instructions[:] = new_insts


def build_nc():
    nc = bass.Bass()

    # ---- DRAM I/O ----
    x_in = nc.dram_tensor("x", [C, SPC, Fd], DT.float32, kind="ExternalInput")
    out_d = nc.dram_tensor("out", [C, SPC, Fd], DT.float32,
                           kind="ExternalOutput")
    wqkv2_d = nc.dram_tensor("wqkv2", [128, 2, 3 * C], DT.bfloat16,
                             kind="ExternalInput")
    corr2_d = nc.dram_tensor("corr2", [2, 3 * C], DT.bfloat16,
                             kind="ExternalInput")
    corrR_d = nc.dram_tensor("corrR", [2, 8 * 256], DT.bfloat16,
                             kind="ExternalInput")
    tapw_d = nc.dram_tensor("tapw", [128, 6, 2], DT.float32,
                            kind="ExternalInput")
    wproj_d = nc.dram_tensor("wproj", [128, 2, C], DT.bfloat16,
                             kind="ExternalInput")
    w1_d = nc.dram_tensor("w1", [128, 2, HID], DT.bfloat16,
                          kind="ExternalInput")
    w1cs_d = nc.dram_tensor("w1cs", [1, HID], DT.bfloat16,
                            kind="ExternalInput")
    w2_d = nc.dram_tensor("w2", [128, 4, C], DT.bfloat16,
                          kind="ExternalInput")
    b1v_d = nc.dram_tensor("b1v", [128, 4], DT.float32, kind="ExternalInput")
    b2v_d = nc.dram_tensor("b2v", [128, 2], DT.float32, kind="ExternalInput")
    temp_d = nc.dram_tensor("temp128", [128, 1], DT.float32,
                            kind="ExternalInput")
    onesb_d = nc.dram_tensor("onesb", [128, 128], DT.bfloat16,
                             kind="ExternalInput")
    bandh0_d = nc.dram_tensor("bandh0", [128, 128], DT.bfloat16,
                              kind="ExternalInput")
    bandh1_d = nc.dram_tensor("bandh1", [128, 128], DT.bfloat16,
                              kind="ExternalInput")
    rowind4_d = nc.dram_tensor("rowind4", [4, 128], DT.bfloat16,
                               kind="ExternalInput")
    ident_d = nc.dram_tensor("identb", [128, 128], DT.bfloat16,
                             kind="ExternalInput")


    with _TileContext(nc) as tc, contextlib.ExitStack() as ctx:
        cpool = ctx.enter_context(tc.tile_pool(name="consts", bufs=1))
        xpool = ctx.enter_context(tc.tile_pool(name="xp", bufs=2))
        xrpool = ctx.enter_context(tc.tile_pool(name="xrp", bufs=2))
        sqp = ctx.enter_context(tc.tile_pool(name="sqp", bufs=2))
        qkp = ctx.enter_context(tc.tile_pool(name="qkp", bufs=6))
        vvp = ctx.enter_context(tc.tile_pool(name="vvp", bufs=2))
        vpp = ctx.enter_context(tc.tile_pool(name="vpp", bufs=2))
        phap = ctx.enter_context(tc.tile_pool(name="phap", bufs=3))
        ostp = ctx.enter_context(tc.tile_pool(name="ostp", bufs=3))
        ohp = ctx.enter_context(tc.tile_pool(name="ohp", bufs=2))
        o1p = ctx.enter_context(tc.tile_pool(name="o1p", bufs=2))
        xhp = ctx.enter_context(tc.tile_pool(name="xhp", bufs=2))
        gelp = ctx.enter_context(tc.tile_pool(name="gelp", bufs=4))
        o2p = ctx.enter_context(tc.tile_pool(name="o2p", bufs=2))
        rows = ctx.enter_context(tc.tile_pool(name="rows", bufs=4))
        dense = ctx.enter_context(tc.tile_pool(name="dense", bufs=1))
        ps = ctx.enter_context(tc.tile_pool(name="ps", bufs=4, space="PSUM"))

        # ---- constants ----
        def cload(name, shape, dt, dram):
            t = cpool.tile(shape, dt, tag=name, name=name)
            nc.sync.dma_start(t[:], dram[:])
            return t

        wqkv_sb = cload("wqkv", [128, 2, 3 * C], DT.bfloat16, wqkv2_d)
        corr2_sb = cload("corr2", [2, 3 * C], DT.bfloat16, corr2_d)
        tapw = cload("tapw", [128, 6, 2], DT.float32, tapw_d)
        wproj_sb = cload("wproj", [128, 2, C], DT.bfloat16, wproj_d)
        w1_sb = cload("w1", [128, 2, HID], DT.bfloat16, w1_d)
        w1cs_sb = cload("w1cs", [1, HID], DT.bfloat16, w1cs_d)
        w2_sb = cload("w2", [128, 4, C], DT.bfloat16, w2_d)
        b1v = cload("b1v", [128, 4], DT.float32, b1v_d)
        b2v = cload("b2v", [128, 2], DT.float32, b2v_d)
        temp128 = cload("temp", [128, 1], DT.float32, temp_d)
        onesb = cload("onesb", [128, 128], DT.bfloat16, onesb_d)
        bandh = [cload("bandh0", [128, 128], DT.bfloat16, bandh0_d),
                 cload("bandh1", [128, 128], DT.bfloat16, bandh1_d)]
        rowind4 = cload("rowind4", [4, 128], DT.bfloat16, rowind4_d)
        identb = cload("identb", [128, 128], DT.bfloat16, ident_d)

        # xn: static pair, plain layout (conv guards live in qpre now)
        xn_st = [[cpool.tile([128, W], DT.bfloat16,
                             name=f"xn{i}_{kt}", tag=f"xn{i}_{kt}")
                  for kt in range(2)] for i in range(1)]
        # qpre: guarded staging for the depthwise conv (zero guard cols once)
        qpre_st = [cpool.tile([128, S, PX], DT.bfloat16,
                              name=f"qpre{i}", tag=f"qpre{i}")
                   for i in range(3)]
        for i in range(3):
            nc.vector.memset(qpre_st[i][:, :, 0:1], 0.0)
            nc.vector.memset(qpre_st[i][:, :, PX - 1:PX], 0.0)

        # ---- dense helpers (gpsimd newton chains, [*,128] tiles) ----
        def rsqrt_dense(x_ap, out_ap, iters=2, eng=None):
            eng = eng or nc.gpsimd
            shape = list(x_ap.shape)
            s1 = dense.tile(shape, DT.int32, tag="nw_i1", name="nw_i1")
            nc.vector.tensor_scalar(s1[:], x_ap.bitcast(DT.int32), 1, None,
                                    ALU.arith_shift_right)
            nc.vector.tensor_scalar(s1[:], s1[:], -1, None, ALU.bitwise_xor)
            nc.vector.tensor_scalar(s1[:], s1[:], RSQRT_MAGIC + 1, None,
                                    ALU.add)
            y = s1[:].bitcast(DT.float32)
            for it in range(iters):
                t = dense.tile(shape, DT.float32, tag="nw_t", name="nw_t")
                u = dense.tile(shape, DT.float32, tag="nw_u", name="nw_u")
                eng.tensor_mul(t[:], y, y)
                eng.tensor_scalar(t[:], t[:], -0.5, None, ALU.mult)
                eng.tensor_mul(u[:], t[:], x_ap)
                eng.tensor_scalar(u[:], u[:], 1.5, None, ALU.add)
                last = (it == iters - 1)
                ynew = out_ap if last else dense.tile(
                    shape, DT.float32, name="nw_y", tag="nw_y", bufs=2)
                yap = ynew if last else ynew[:]
                eng.tensor_mul(yap, u[:], y)
                y = yap

        def recip_dense(x_ap, out_ap, iters=2, eng=None):
            eng = eng or nc.gpsimd
            shape = list(x_ap.shape)
            s1 = dense.tile(shape, DT.int32, tag="nw_i1", name="nw_i1")
            nc.vector.tensor_scalar(s1[:], x_ap.bitcast(DT.int32), -1, None,
                                    ALU.bitwise_xor)
            nc.vector.tensor_scalar(s1[:], s1[:], 0x7EF127EA + 1, None,
                                    ALU.add)
            y = s1[:].bitcast(DT.float32)
            for it in range(iters):
                u = dense.tile(shape, DT.float32, tag="nw_t", name="nw_t")
                eng.tensor_mul(u[:], x_ap, y)
                v = dense.tile(shape, DT.float32, tag="nw_u", name="nw_u")
                eng.tensor_scalar(v[:], u[:], -1.0, None, ALU.mult)
                eng.tensor_scalar(v[:], v[:], 2.0, None, ALU.add)
                last = (it == iters - 1)
                ynew = out_ap if last else dense.tile(
                    shape, DT.float32, name="nw_y", tag="nw_y", bufs=2)
                yap = ynew if last else ynew[:]
                eng.tensor_mul(yap, v[:], y)
                y = yap

        st = {b: {} for b in range(NBLK)}

        # ---------------- stages ----------------
        def s0_load(blk):
            d = st[blk]
            d["xb"] = [sqp.tile([128, W], DT.bfloat16, name=f"xb_{kt}",
                                tag="xb") for kt in range(2)]
            for kt in range(2):
                for h2 in range(2):
                    xh32 = xpool.tile([128, S // 2, Fd], DT.float32,
                                      name="xh32", tag="x32")
                    nc.sync.dma_start(
                        xh32[:],
                        x_in[kt * 128:(kt + 1) * 128,
                             blk * S + h2 * 4:blk * S + (h2 + 1) * 4, :])
                    nc.vector.tensor_copy(
                        d["xb"][kt][:, h2 * HW_:(h2 + 1) * HW_],
                        xh32[:].rearrange("p s f -> p (s f)"))

        def _stats(blk, srcs, sqs, which, fr):
            """Partition-sum stats of srcs (and sqs) -> dense mu*rsig and
            rsig rows (bf16 [16,128]) stored as mr{which}/rr{which}."""
            d = st[blk]
            su_row = rows.tile([1, W], DT.bfloat16,
                               name=f"su_{which}", tag="rows")
            sq_row = rows.tile([1, W], DT.bfloat16,
                               name=f"sqr_{which}", tag="rows")
            for h2 in range(2):
                ps_su = ps.tile([1, HW_], DT.float32, tag="mm", name="ps_su")
                ps_sq = ps.tile([1, HW_], DT.float32, tag="mm", name="ps_sq")
                for ch in range(2):
                    sl = slice(h2 * HW_ + ch * 512, h2 * HW_ + (ch + 1) * 512)
                    psl = slice(ch * 512, (ch + 1) * 512)
                    for kt in range(2):
                        nc.tensor.matmul(
                            ps_su[0:1, psl], onesb[:, 0:1],
                            srcs[kt][:, sl],
                            start=(kt == 0), stop=(kt == 1),
                            skip_group_check=True)
                    for kt in range(2):
                        nc.tensor.matmul(
                            ps_sq[0:1, psl], onesb[:, 0:1],
                            sqs[(kt, h2)][:, psl],
                            start=(kt == 0), stop=(kt == 1),
                            skip_group_check=True)
                nc.scalar.activation(su_row[0:1, h2 * HW_:(h2 + 1) * HW_],
                                     ps_su[:], AF.Copy)
                nc.scalar.activation(sq_row[0:1, h2 * HW_:(h2 + 1) * HW_],
                                     ps_sq[:], AF.Copy)
            dsu = dense.tile([16, 128], DT.bfloat16, tag="dsu", name="dsu")
            dsq = dense.tile([16, 128], DT.bfloat16, tag="dsq", name="dsq")
            nc.sync.dma_start(
                dsu[:], su_row[:].rearrange("o (j c) -> o j c", c=128))
            nc.sync.dma_start(
                dsq[:], sq_row[:].rearrange("o (j c) -> o j c", c=128))
            mu = dense.tile([16, 128], DT.float32, tag="dmu", name="dmu")
            nc.vector.tensor_scalar(mu[:], dsu[:], 1.0 / C, None, ALU.mult)
            var = dense.tile([16, 128], DT.float32, tag="dvar", name="dvar")
            m2 = dense.tile([16, 128], DT.float32, tag="nw_t", name="dm2")
            nc.vector.tensor_mul(m2[:], mu[:], mu[:])
            nc.vector.tensor_scalar(var[:], dsq[:], 1.0 / C, LN_EPS,
                                    ALU.mult, ALU.add)
            nc.vector.tensor_sub(var[:], var[:], m2[:])
            rsd = dense.tile([16, 128], DT.float32, tag="drs", name="drs")
            rsqrt_dense(var[:], rsd[:], iters=1, eng=nc.vector)
            mr = dense.tile([16, 128], DT.bfloat16, tag="dmr", name="dmr")
            nc.vector.tensor_mul(mr[:], mu[:], rsd[:])
            rr = dense.tile([16, 128], DT.bfloat16, tag="drr", name="drr")
            nc.vector.tensor_copy(rr[:], rsd[:])
            d[f"mr{which}"] = mr
            d[f"rr{which}"] = rr

        def s1_ln1_stats(blk):
            d = st[blk]
            srcs = [d["xb"][kt][:] for kt in range(2)]
            sqs = {}
            for kt in range(2):
                for h2 in range(2):
                    x2 = sqp.tile([128, HW_], DT.bfloat16, name="x2",
                                  tag="sqh")
                    s_ = srcs[kt][:, h2 * HW_:(h2 + 1) * HW_]
                    nc.vector.scalar_tensor_tensor(
                        x2[:], s_, 1.0, s_, ALU.mult, ALU.mult)
                    sqs[(kt, h2)] = x2
            _stats(blk, srcs, sqs, 1, fr=False)

        def s2_ln1_apply(blk):
            d = st[blk]
            mr, rr = d.pop("mr1"), d.pop("rr1")
            rrow = rows.tile([1, W], DT.bfloat16, name="rrow", tag="rows")
            nc.sync.dma_start(
                rrow[:].rearrange("o (j c) -> o j c", c=128), rr[:])
            corrR = rows.tile([2, W], DT.bfloat16, name="corrR", tag="rows")
            nc.sync.dma_start(
                corrR[0:1, :].rearrange("o (j c) -> o j c", c=128), mr[:])
            nc.sync.dma_start(corrR[1:2, :], corrR_d[1:2, :])
            d["corrR"] = corrR
            xn16 = xn_st[0]
            for h2 in range(2):
                ps_r = ps.tile([128, HW_], DT.float32, tag="mm", name="ps_r")
                for ch in range(2):
                    nc.tensor.matmul(
                        ps_r[:, ch * 512:(ch + 1) * 512],
                        onesb[0:1, :],
                        rrow[0:1, h2 * HW_ + ch * 512:
                             h2 * HW_ + (ch + 1) * 512],
                        start=True, stop=True, skip_group_check=True)
                for kt in range(2):
                    nc.vector.tensor_mul(
                        xn16[kt][:, h2 * HW_:(h2 + 1) * HW_],
                        d["xb"][kt][:, h2 * HW_:(h2 + 1) * HW_],
                        ps_r[:])
            d["xn"] = xn16

        def qkv_m(blk, m, qc):
            d = st[blk]
            qp3 = qpre_st[m % 3][:]
            if m < 4:
                qt = qkp.tile([128, W], DT.bfloat16, name=f"qc{m}", tag="qk")
            else:
                qt = vvp.tile([128, W], DT.bfloat16, name=f"vc{m}", tag="vv")
            qc.append(qt)
            for h2 in range(2):
                ps_m = ps.tile([128, HW_], DT.float32, tag="mm", name="ps_m")
                for ch in range(2):
                    sl = slice(h2 * HW_ + ch * 512, h2 * HW_ + (ch + 1) * 512)
                    for kt in range(2):
                        nc.tensor.matmul(
                            ps_m[:, ch * 512:(ch + 1) * 512],
                            wqkv_sb[:, kt, m * 128:(m + 1) * 128],
                            d["xn"][kt][:, sl],
                            start=(kt == 0), stop=False,
                            skip_group_check=True)
                    nc.tensor.matmul(
                        ps_m[:, ch * 512:(ch + 1) * 512],
                        corr2_sb[:, m * 128:(m + 1) * 128],
                        d["corrR"][:, sl],
                        start=False, stop=True, skip_group_check=True)
                nc.vector.tensor_copy(
                    qp3[:, h2 * 4:(h2 + 1) * 4, 1:1 + Fd],
                    ps_m[:].rearrange("p (s f) -> p s f", f=Fd))
            # depthwise conv3 along f (middle tap folded into Wqkv):
            # qc = qpre + r0*shift(-1) + r2*shift(+1), in place (bf16 DVE)
            vm1 = qp3[:, :, 0:Fd]
            v00 = qp3[:, :, 1:1 + Fd]
            vp1 = qp3[:, :, 2:2 + Fd]
            qf = qc[m][:].rearrange("p (s f) -> p s f", f=Fd)
            nc.vector.scalar_tensor_tensor(qf, vm1, tapw[:, m, 0:1], v00,
                                           ALU.mult, ALU.add)
            nc.vector.scalar_tensor_tensor(qf, vp1, tapw[:, m, 1:2], qf,
                                           ALU.mult, ALU.add)

        def qkv_tail(blk, qc):
            d = st[blk]
            d.pop("corrR")
            d["qc"] = qc[:4]
            # v transpose (frees v tiles fast)
            vp = [vpp.tile([128, S, H, Dh + 1], DT.bfloat16,
                           name=f"vp{b2}", tag=f"vp{b2}") for b2 in range(2)]
            for b2 in range(2):
                nc.vector.memset(vp[b2][:, :, :, Dh:Dh + 1], 1.0)
            for ti in range(2):
                vt = qc[4 + ti]
                for b2 in range(2):
                    ps_tp = ps.tile([128, S * 128], DT.bfloat16, tag="mm",
                                    name="ps_tp")
                    for s in range(S):
                        nc.tensor.transpose(
                            ps_tp[:, s * 128:(s + 1) * 128],
                            vt[:, s * Fd + b2 * 128:s * Fd + b2 * 128 + 128],
                            identb[:])
                    nc.vector.tensor_copy(
                        vp[b2][:, :, 4 * ti:4 * ti + 4, 0:Dh],
                        ps_tp[:].rearrange("p (s hb d) -> p s hb d",
                                           s=S, hb=4))
            d["vp"] = vp

        def s3_qkv(blk):
            qc = []
            for m in range(6):
                qkv_m(blk, m, qc)
            qkv_tail(blk, qc)

        def s4_l2sumsq(blk):
            d = st[blk]
            d["dnq"] = {}
            for vi, base in (("q", 0), ("k", 2)):
                d_n = dense.tile([128, 128], DT.bfloat16, tag="dn", name="dn")
                for h2 in range(2):
                    ps_n = ps.tile([128, HW_], DT.float32, tag="mm",
                                   name="ps_n")
                    for ti in range(2):
                        sq = sqp.tile([128, HW_], DT.bfloat16, name="l2sq",
                                      tag="sqh")
                        qs = d["qc"][base + ti][:, h2 * HW_:(h2 + 1) * HW_]
                        nc.scalar.activation(sq[:], qs, AF.Square)
                        for ch in range(2):
                            nc.tensor.matmul(
                                ps_n[:, ch * 512:(ch + 1) * 512],
                                bandh[ti][:], sq[:, ch * 512:(ch + 1) * 512],
                                start=(ti == 0), stop=(ti == 1),
                                skip_group_check=True)
                    nsb = rows.tile([8, HW_], DT.bfloat16, name="nsb",
                                    tag="rows")
                    nc.any.tensor_copy(nsb[:], ps_n[0:8, :])
                    nc.sync.dma_start(
                        d_n[h2 * 64:(h2 + 1) * 64, :],
                        nsb[:].rearrange("h (j c) -> h j c", c=128))
                dnf = dense.tile([128, 128], DT.float32, tag="dnf",
                                 name="dnf")
                nc.vector.tensor_copy(dnf[:], d_n[:])
                r_n = dense.tile([128, 128], DT.float32, tag="dr", name="dr")
                rsqrt_dense(dnf[:], r_n[:], iters=1, eng=nc.vector)
                r16 = dense.tile([128, 128], DT.bfloat16, tag="dr16",
                                 name="dr16")
                if vi == "k":
                    nc.vector.tensor_scalar(r16[:], r_n[:], temp128[:, 0:1],
                                            None, ALU.mult)
                else:
                    nc.vector.tensor_copy(r16[:], r_n[:])
                d["dnq"][vi] = r16

        def s5_l2apply(blk):
            d = st[blk]
            for vi, base in (("q", 0), ("k", 2)):
                r16 = d["dnq"].pop(vi)
                for ti in range(2):
                    r4 = rows.tile([4, W], DT.bfloat16, name="r4", tag="rows")
                    for h2 in range(2):
                        nc.sync.dma_start(
                            r4[:, h2 * HW_:(h2 + 1) * HW_].rearrange(
                                "b (j c) -> b j c", c=128),
                            r16[h2 * 64 + ti * 32:h2 * 64 + ti * 32 + 32, :])
                    for h2 in range(2):
                        ps_b = ps.tile([128, HW_], DT.float32, tag="mm",
                                       name="ps_b")
                        for ch in range(2):
                            sl = slice(h2 * HW_ + ch * 512,
                                       h2 * HW_ + (ch + 1) * 512)
                            nc.tensor.matmul(
                                ps_b[:, ch * 512:(ch + 1) * 512],
                                rowind4[:], r4[:, sl],
                                start=True, stop=True, skip_group_check=True)
                        qs = d["qc"][base + ti][:, h2 * HW_:(h2 + 1) * HW_]
                        nc.vector.tensor_mul(qs, qs, ps_b[:])
            d.pop("dnq")

        def attn_h(blk, h, oh16, d_rs):
            d = st[blk]
            ti, band = h // 4, (h % 4) * 32
            phat = []
            for jt in range(2):
                pj = phap.tile([128, W], DT.bfloat16, tag="phat",
                               name="phat")
                for h2 in range(2):
                    ps_S = ps.tile([128, HW_], DT.float32, tag="mm",
                                   name="ps_S")
                    for si in range(4):
                        s = h2 * 4 + si
                        nc.tensor.matmul(
                            ps_S[:, si * Fd:(si + 1) * Fd],
                            d["qc"][2 + ti][
                                band:band + 32,
                                s * Fd + jt * 128:s * Fd + jt * 128 + 128],
                            d["qc"][ti][band:band + 32,
                                        s * Fd:(s + 1) * Fd],
                            start=True, stop=True, skip_group_check=True,
                            tile_position=(band, 0))
                    nc.scalar.activation(
                        pj[:, h2 * HW_:(h2 + 1) * HW_], ps_S[:], AF.Exp)
                phat.append(pj)
            ostg = ostp.tile([Dh + 1, W], DT.bfloat16, tag="ostg",
                             name="ostg")
            for h2 in range(2):
                ps_O = ps.tile([Dh + 1, HW_], DT.float32, tag="mm",
                               name="ps_O")
                for si in range(4):
                    s = h2 * 4 + si
                    for ktj in range(2):
                        nc.tensor.matmul(
                            ps_O[:, si * Fd:(si + 1) * Fd],
                            d["vp"][ktj][:, s, h, :],
                            phat[ktj][:, s * Fd:(s + 1) * Fd],
                            start=(ktj == 0), stop=(ktj == 1),
                            skip_group_check=True)
                nc.any.tensor_copy(ostg[:, h2 * HW_:(h2 + 1) * HW_],
                                   ps_O[:])
            nc.gpsimd.dma_start(oh16[ti][band:band + 32, :],
                                ostg[0:Dh, :])
            nc.sync.dma_start(
                d_rs[h * 16:(h + 1) * 16, :],
                ostg[Dh:Dh + 1, :].rearrange("o (j c) -> o j c", c=128))

        def attn_head_setup(blk):
            d = st[blk]
            oh16 = [ohp.tile([128, W], DT.bfloat16, name=f"oh{ti}", tag="oh")
                    for ti in range(2)]
            d_rs = dense.tile([128, 128], DT.bfloat16, tag="dnr", name="dnr")
            d["oh16"] = oh16
            return oh16, d_rs

        def attn_tail(blk, d_rs):
            d = st[blk]
            drf = dense.tile([128, 128], DT.float32, tag="drf", name="drf")
            nc.vector.tensor_copy(drf[:], d_rs[:])
            d_ri = dense.tile([128, 128], DT.bfloat16, tag="dri", name="dri")
            recip_dense(drf[:], d_ri[:], eng=nc.vector)
            d["d_ri"] = d_ri

        def s6_attn(blk):
            oh16, d_rs = attn_head_setup(blk)
            for h in range(H):
                attn_h(blk, h, oh16, d_rs)
            attn_tail(blk, d_rs)

        def merged_attn_qkv(ba, bq):
            """Interleave attention(ba) heads with qkv(bq) m-tiles so the
            shared psum ring rotates through both stages."""
            oh16, d_rs = attn_head_setup(ba)
            qc = []
            plan = ["h0", "h1", "m0", "h2", "m1", "h3", "m2", "h4", "m3",
                    "h5", "m4", "h6", "m5", "h7"]
            for step in plan:
                if step[0] == "h":
                    attn_h(ba, int(step[1]), oh16, d_rs)
                else:
                    qkv_m(bq, int(step[1]), qc)
            qkv_tail(bq, qc)
            attn_tail(ba, d_rs)

        def s7_proj(blk):
            d = st[blk]
            d_ri = d.pop("d_ri")
            for ti in range(2):
                r4 = rows.tile([4, W], DT.bfloat16, name="rinv", tag="rows")
                nc.sync.dma_start(
                    r4[:].rearrange("b (j c) -> b j c", c=128),
                    d_ri[ti * 64:(ti + 1) * 64, :])
                for h2 in range(2):
                    ps_b = ps.tile([128, HW_], DT.float32, tag="mm",
                                   name="ps_b2")
                    for ch in range(2):
                        sl = slice(h2 * HW_ + ch * 512,
                                   h2 * HW_ + (ch + 1) * 512)
                        nc.tensor.matmul(
                            ps_b[:, ch * 512:(ch + 1) * 512],
                            rowind4[:], r4[:, sl],
                            start=True, stop=True, skip_group_check=True)
                    ohs = d["oh16"][ti][:, h2 * HW_:(h2 + 1) * HW_]
                    nc.vector.tensor_mul(ohs, ohs, ps_b[:])
            o1 = [o1p.tile([128, W], DT.bfloat16, name=f"o1_{m}", tag="o1")
                  for m in range(2)]
            for m2 in range(2):
                for h2 in range(2):
                    xr32 = xrpool.tile([128, S // 2, Fd], DT.float32,
                                       name="xr32", tag="xr32")
                    nc.sync.dma_start(
                        xr32[:],
                        x_in[m2 * 128:(m2 + 1) * 128,
                             blk * S + h2 * 4:blk * S + (h2 + 1) * 4, :])
                    xf = xr32[:].rearrange("p s f -> p (s f)")
                    ps_y = ps.tile([128, HW_], DT.float32, tag="mm",
                                   name="ps_y")
                    for ch in range(2):
                        sl = slice(h2 * HW_ + ch * 512,
                                   h2 * HW_ + (ch + 1) * 512)
                        for kt in range(2):
                            nc.tensor.matmul(
                                ps_y[:, ch * 512:(ch + 1) * 512],
                                wproj_sb[:, kt, m2 * 128:(m2 + 1) * 128],
                                d["oh16"][kt][:, sl],
                                start=(kt == 0), stop=(kt == 1),
                                skip_group_check=True)
                    nc.vector.scalar_tensor_tensor(
                        o1[m2][:, h2 * HW_:(h2 + 1) * HW_],
                        ps_y[:], 1.0, xf[:], ALU.mult, ALU.add)
            d["o1"] = o1
            d.pop("oh16")
            d.pop("qc")
            d.pop("vp")
            d.pop("xn")
            d.pop("xb")

        def s8_ln2_stats(blk):
            d = st[blk]
            srcs = [d["o1"][kt][:] for kt in range(2)]
            sqs = {}
            for kt in range(2):
                for h2 in range(2):
                    x2 = sqp.tile([128, HW_], DT.bfloat16, name="o1sq",
                                  tag="sqh")
                    s_ = srcs[kt][:, h2 * HW_:(h2 + 1) * HW_]
                    nc.scalar.activation(x2[:], s_, AF.Square)
                    sqs[(kt, h2)] = x2
            _stats(blk, srcs, sqs, 2, fr=False)

        def mlp_head(blk):
            d = st[blk]
            mr, rr = d.pop("mr2"), d.pop("rr2")
            rrow = rows.tile([1, W], DT.bfloat16, name="rrow2", tag="rows")
            nc.sync.dma_start(
                rrow[:].rearrange("o (j c) -> o j c", c=128), rr[:])
            mrow = rows.tile([1, W], DT.bfloat16, name="mrow2", tag="rows")
            nc.sync.dma_start(
                mrow[:].rearrange("o (j c) -> o j c", c=128), mr[:])
            xh = [xhp.tile([128, W], DT.bfloat16, name=f"xh{kt}", tag="xh")
                  for kt in range(2)]
            for h2 in range(2):
                ps_r = ps.tile([128, HW_], DT.float32, tag="mm", name="ps_r2")
                for ch in range(2):
                    nc.tensor.matmul(
                        ps_r[:, ch * 512:(ch + 1) * 512],
                        onesb[0:1, :],
                        rrow[0:1, h2 * HW_ + ch * 512:
                             h2 * HW_ + (ch + 1) * 512],
                        start=True, stop=True, skip_group_check=True)
                for kt in range(2):
                    nc.vector.tensor_mul(
                        xh[kt][:, h2 * HW_:(h2 + 1) * HW_],
                        d["o1"][kt][:, h2 * HW_:(h2 + 1) * HW_],
                        ps_r[:])
            g16 = [gelp.tile([128, W], DT.bfloat16, name=f"gel{m}", tag="gel")
                   for m in range(4)]
            d["xh"] = xh
            d["mrow"] = mrow
            d["g16"] = g16

        def mlp_w1(blk, mh):
            d = st[blk]
            xh, mrow, g16 = d["xh"], d["mrow"], d["g16"]
            for h2 in range(2):
                ps_h = ps.tile([128, HW_], DT.float32, tag="mm",
                               name="ps_h")
                for ch in range(2):
                    sl = slice(h2 * HW_ + ch * 512,
                               h2 * HW_ + (ch + 1) * 512)
                    for kt in range(2):
                        nc.tensor.matmul(
                            ps_h[:, ch * 512:(ch + 1) * 512],
                            w1_sb[:, kt, mh * 128:(mh + 1) * 128],
                            xh[kt][:, sl],
                            start=(kt == 0), stop=False,
                            skip_group_check=True)
                    nc.tensor.matmul(
                        ps_h[:, ch * 512:(ch + 1) * 512],
                        w1cs_sb[0:1, mh * 128:(mh + 1) * 128],
                        mrow[0:1, sl],
                        start=False, stop=True, skip_group_check=True)
                nc.scalar.activation(
                    g16[mh][:, h2 * HW_:(h2 + 1) * HW_], ps_h[:],
                    AF.Gelu, bias=b1v[:, mh:mh + 1], scale=1.0)

        def mlp_w2(blk, m2, h2):
            d = st[blk]
            g16 = d["g16"]
            ps_y = ps.tile([128, HW_], DT.float32, tag="mm", name="ps_y2")
            for ch in range(2):
                for kt in range(4):
                    nc.tensor.matmul(
                        ps_y[:, ch * 512:(ch + 1) * 512],
                        w2_sb[:, kt, m2 * 128:(m2 + 1) * 128],
                        g16[kt][:, h2 * HW_ + ch * 512:
                                h2 * HW_ + (ch + 1) * 512],
                        start=(kt == 0), stop=(kt == 3),
                        skip_group_check=True)
            o2 = o2p.tile([128, HW_], DT.float32, tag="o2", name="o2t")
            nc.vector.scalar_tensor_tensor(
                o2[:], ps_y[:], b2v[:, m2:m2 + 1],
                d["o1"][m2][:, h2 * HW_:(h2 + 1) * HW_],
                ALU.add, ALU.add)
            nc.sync.dma_start(
                out_d[m2 * 128:(m2 + 1) * 128,
                      blk * S + h2 * 4:blk * S + (h2 + 1) * 4, :],
                o2[:].rearrange("p (s f) -> p s f", f=Fd))

        def mlp_tail(blk):
            d = st[blk]
            d.pop("xh")
            d.pop("mrow")
            d.pop("g16")
            d.pop("o1")

        def s9_mlp(blk):
            mlp_head(blk)
            for mh in range(4):
                mlp_w1(blk, mh)
            for m2 in range(2):
                for h2 in range(2):
                    mlp_w2(blk, m2, h2)
            mlp_tail(blk)

        def merged_step(bm, ba, bq):
            """Interleave mlp(bm), attention(ba), qkv(bq); any may be None."""
            if ba is not None:
                oh16, d_rs = attn_head_setup(ba)
            if bm is not None:
                mlp_head(bm)
            qc = []
            plan = []
            for i in range(8):
                if ba is not None:
                    plan.append(("h", i))
                if bm is not None and i < 4:
                    plan.append(("w1", i))
                if bm is not None and 4 <= i < 8:
                    plan.append(("w2", i - 4))
                if bq is not None and 1 <= i < 7:
                    plan.append(("m", i - 1))
            for kind, i in plan:
                if kind == "h":
                    attn_h(ba, i, oh16, d_rs)
                elif kind == "w1":
                    mlp_w1(bm, i)
                elif kind == "w2":
                    mlp_w2(bm, i // 2, i % 2)
                else:
                    qkv_m(bq, i, qc)
            if bq is not None:
                qkv_tail(bq, qc)
            if ba is not None:
                attn_tail(ba, d_rs)
            if bm is not None:
                mlp_tail(bm)

        stages = [s0_load, s1_ln1_stats, s2_ln1_apply, s3_qkv, s4_l2sumsq,
                  s5_l2apply, s6_attn, s7_proj, s8_ln2_stats, s9_mlp]


        # skew-3 wavefront: later stages (lower block index) first.
        # s9(b-1), s6(b), s3(b+1) land on the same step; emit them
        # interleaved so the shared psum ring rotates through all three.
        nst = len(stages)
        for t in range(nst + SKEW * (NBLK - 1)):
            todo = [(b, t - SKEW * b) for b in range(NBLK)
                    if 0 <= t - SKEW * b < nst]
            jmap = {j: b for (b, j) in todo}
            skip = set()
            if 6 in jmap:
                bm = jmap.get(9)
                ba = jmap[6]
                bq = jmap.get(3)
                for j in (9, 6, 3):
                    if j in jmap:
                        skip.add((jmap[j], j))
            order = sorted(todo, key=lambda bj: (0 if bj[1] == 4 else 1,
                                                 bj[0]))
            for b, j in order:
                if (b, j) in skip:
                    if j == 6:
                        merged_step(jmap.get(9), b, jmap.get(3))
                    continue
                stages[j](b)

    _split_excess_waits(nc, max_waits=1)
    return nc


def _host_prep(inputs):
    Wqkv = np.asarray(inputs["Wqkv"], np.float64)        # (C, 3C)
    g1 = np.asarray(inputs["norm1_g"], np.float64)
    b1n = np.asarray(inputs["norm1_b"], np.float64)
    g2 = np.asarray(inputs["norm2_g"], np.float64)
    b2n = np.asarray(inputs["norm2_b"], np.float64)
    dw_w = np.asarray(inputs["dw_w"], np.float64)
    taps = dw_w[:, 0, :]                                 # (3C, 3)

    Wq = Wqkv * g1[:, None]                              # g1 folded
    # fold the middle conv tap into the weights; conv uses tap ratios
    w1t = taps[:, 1].copy()
    w1t = np.where(np.abs(w1t) < 1e-30, 1e-30, w1t)
    Wqf = Wq * w1t[None, :]
    wqkv2 = np.ascontiguousarray(
        Wqf.reshape(2, 128, 3 * C).transpose(1, 0, 2)).astype(BF16)

    colsum = Wqf.sum(axis=0)                             # (3C,)
    bq = (b1n @ Wqkv) * w1t                              # (3C,)
    corr2 = np.stack([-colsum, bq]).astype(BF16)         # (2, 3C)
    corrR = np.zeros((2, S * Fd), np.float32)
    corrR[1] = 1.0                                       # static ones row
    ratios = np.stack([taps[:, 0] / w1t, taps[:, 2] / w1t], axis=1)
    tapw = np.ascontiguousarray(
        ratios.reshape(6, 128, 2).transpose(1, 0, 2)).astype(np.float32)

    def kt_major(w, nkt):
        K, N = w.shape
        return np.ascontiguousarray(
            w.reshape(nkt, 128, N).transpose(1, 0, 2)).astype(BF16)

    wproj = kt_major(np.asarray(inputs["Wproj"], np.float64), 2)
    W1 = np.asarray(inputs["W1"], np.float64)
    W1g = W1 * g2[:, None]
    w1 = kt_major(W1g, 2)
    w1cs = (-W1g.sum(axis=0)).reshape(1, HID).astype(BF16)
    w2 = kt_major(np.asarray(inputs["W2"], np.float64), 4)

    b1p = np.asarray(inputs["b1"], np.float64) + b2n @ W1
    b1v = np.ascontiguousarray(b1p.reshape(4, 128).T).astype(np.float32)
    b2v = np.ascontiguousarray(
        np.asarray(inputs["b2"], np.float32).reshape(2, 128).T)

    temp = np.asarray(inputs["temperature"], np.float32).reshape(H)
    # l2 dense rows are h2-major: row = h2*64 + head*8 + j
    temp128 = np.array([temp[(r % 64) // 8] for r in range(128)],
                       np.float32).reshape(128, 1)

    bandh0 = np.zeros((128, 128), np.float32)
    bandh1 = np.zeros((128, 128), np.float32)
    for dd in range(128):
        for m in range(128):
            if m % 8 == dd // 32:
                bandh0[dd, m] = 1.0
            if m % 8 == 4 + dd // 32:
                bandh1[dd, m] = 1.0
    rowind4 = np.zeros((4, 128), np.float32)
    for m in range(128):
        rowind4[m // 32, m] = 1.0

    return dict(
        wqkv2=wqkv2, corr2=corr2, corrR=corrR.astype(BF16), tapw=tapw,
        wproj=wproj, w1=w1, w1cs=w1cs, w2=w2,
        b1v=b1v, b2v=b2v, temp128=temp128,
        onesb=np.ones((128, 128), BF16),
        bandh0=bandh0.astype(BF16),
        bandh1=bandh1.astype(BF16),
        rowind4=rowind4.astype(BF16),
        identb=np.eye(128).astype(BF16),
    )


_NC_CACHE = {}


def get_nc():
    if "nc" not in _NC_CACHE:
        _NC_CACHE["nc"] = build_nc()
    return _NC_CACHE["nc"]


def make_in_maps(inputs):
    consts = _host_prep(inputs)
    x = np.asarray(inputs["x"], np.float32)  # (B, C, T, Fd)
    in_maps = []
    for core in range(NCORES):
        b, t0 = core // 2, (core % 2) * SPC
        m = dict(consts)
        m["x"] = np.ascontiguousarray(x[b, :, t0:t0 + SPC, :])
        in_maps.append(m)
    return in_maps


def assemble_out(results):
    out = np.zeros((B, C, T, Fd), np.float32)
    for core in range(NCORES):
        b, t0 = core // 2, (core % 2) * SPC
        out[b, :, t0:t0 + SPC, :] = results[core]["out"]
    return out


def kernel(**inputs):
    nc = get_nc()
    in_maps = make_in_maps(inputs)
    res = run_bass_kernel_spmd(nc, in_maps, core_ids=list(range(NCORES)))
    return assemble_out(res.results)


# revision 8
# speedup vs baseline: 1.0461x; 1.0252x over previous
"""Trainium2 Bass kernel v2 for the AxisMDTA dense-transformer block.

x (4, 256, 64, 256) fp32 -> out (4, 256, 64, 256) fp32.
Data-parallel over the 256 (b,t) samples across 8 NeuronCores (32/core).
Channel-major on-chip layout (c on partitions, (sample, f) on free dim).

v2 vs baseline:
- LN gammas folded into following weights host-side; LN bias + mean
  subtraction folded into the qkv / W1 matmuls as rank-k correction
  matmuls (shifted guarded rows handle the depthwise-conv taps exactly).
- Stats matmuls run f32r directly on fp32 x (no bf16 staging copy);
  squares via scalar_tensor_tensor (DVE 2x modes); dense Newton chains
  on gpsimd.
- Uniform [128,1024] PSUM quanta from one 4-deep ring.
- Stage emission is a skew-3 software-pipeline wavefront across the 4
  blocks so the PE stream always has ready work behind a stalled op.
- Guard columns zeroed once; per-block memsets eliminated.
- bf16 trunk (o1); x reloaded from DRAM for the proj residual; band
  moves via SWDGE (gpsimd) to offload HWDGE.
"""

import contextlib

import numpy as np
import ml_dtypes

import concourse.bass as bass
import concourse.mybir as mybir
import concourse.tile as tile
from concourse.vector_clock import ScopedClock
from concourse.bass_utils import run_bass_kernel_spmd

AF = mybir.ActivationFunctionType
ALU = mybir.AluOpType
DT = mybir.dt
BF16 = ml_dtypes.bfloat16

B, C, T, Fd = 4, 256, 64, 256
H, Dh = 8, 32
HID = 512
NCORES = 8
SPC = (B * T) // NCORES          # 32 samples per core
S = 8                            # samples per block
NBLK = SPC // S                  # 4 blocks
W = S * Fd                       # 2048 free columns per block
HW_ = W // 2                     # 1024-wide psum half
P3 = 260                         # corr row pitch (guarded)
PX = 258                         # xn guarded pitch
SKEW = 3
LN_EPS = 1e-5
RSQRT_MAGIC = 0x5F3759DF


class _TileContext(tile.TileContext):
    """Walrus in this container caps sync-wait commands per CTRL-class
    instruction; spread the exit drain's waits across single-wait nops."""

    def _drain_and_barrier(self, tick_clock, wait_clock):
        drain_inst = self.nc.sync.drain()
        wait_clock.add_sem_waits(
            drain_inst.ins, ScopedClock({None: tick_clock.global_clock})
        )
        si = drain_inst.ins.sync_info
        waits = list(si.on_wait or []) if si else []
        if len(waits) > 1:
            si.on_wait = waits[:1]
            for w in waits[1:]:
                n = self.nc.sync.nop(nofuse=True).ins
                n.sync_info = mybir.SyncInfo(on_wait=[w], on_update=[])
        self.nc.all_engine_barrier()
        assert self.sems is not None
        popped = self.nc._tile_sem_poison_stack.pop()
        assert popped is self._sem_poison
        self.nc.clear_and_free_semaphores(list(self.sems.allocated().values()))
        self.nc.all_engine_barrier()


def _f32r(ap):
    return ap.bitcast(DT.float32r)


def _brep(ap, nrep):
    """Insert a stride-0 replication dim after the partition dim."""
    ap.ap.insert(1, [0, nrep])
    return ap


def _split_excess_waits(nc, max_waits=2):
    """Walrus in this container caps sync-wait commands per instruction.
    Move excess waits onto same-engine NoOps inserted just before."""
    for f in nc.m.functions:
        for bb in f.blocks:
            new_insts = []
            for inst in bb.instructions:
                si = inst.sync_info
                waits = list(si.on_wait) if si and si.on_wait else []
                if len(waits) > max_waits:
                    si.on_wait = waits[:max_waits]
                    rest = waits[max_waits:]
                    for i in range(0, len(rest), max_waits):
                        nop = mybir.InstEventSemaphore(
                            name=f"I-ws{nc.next_id()}", ins=[], outs=[])
                        nop.engine = inst.engine
                        nop.sync_info = mybir.SyncInfo(
                            on_wait=rest[i:i + max_waits], on_update=[])
                        nc.register_instruction(nop)
                        new_insts.append(nop)
                new_insts.append(inst)
            bb.instructions[:] = new_insts


def build_nc():
    nc = bass.Bass()

    # ---- DRAM I/O ----
    x_in = nc.dram_tensor("x", [C, SPC, Fd], DT.float32, kind="ExternalInput")
    out_d = nc.dram_tensor("out", [C, SPC, Fd], DT.float32,
                           kind="ExternalOutput")
    wqkv2_d = nc.dram_tensor("wqkv2", [128, 2, 3 * C], DT.bfloat16,
                             kind="ExternalInput")
    corr2_d = nc.dram_tensor("corr2", [2, 3 * C], DT.bfloat16,
                             kind="ExternalInput")
    corrR_d = nc.dram_tensor("corrR", [2, 8 * 256], DT.bfloat16,
                             kind="ExternalInput")
    tapw_d = nc.dram_tensor("tapw", [128, 6, 2], DT.float32,
                            kind="ExternalInput")
    wproj_d = nc.dram_tensor("wproj", [128, 2, C], DT.bfloat16,
                             kind="ExternalInput")
    w1_d = nc.dram_tensor("w1", [128, 2, HID], DT.bfloat16,
                          kind="ExternalInput")
    w1cs_d = nc.dram_tensor("w1cs", [1, HID], DT.bfloat16,
                            kind="ExternalInput")
    w2_d = nc.dram_tensor("w2", [128, 4, C], DT.bfloat16,
                          kind="ExternalInput")
    b1v_d = nc.dram_tensor("b1v", [128, 4], DT.float32, kind="ExternalInput")
    b2v_d = nc.dram_tensor("b2v", [128, 2], DT.float32, kind="ExternalInput")
    temp_d = nc.dram_tensor("temp128", [128, 1], DT.float32,
                            kind="ExternalInput")
    onesb_d = nc.dram_tensor("onesb", [128, 128], DT.bfloat16,
                             kind="ExternalInput")
    bandh0_d = nc.dram_tensor("bandh0", [128, 128], DT.bfloat16,
                              kind="ExternalInput")
    bandh1_d = nc.dram_tensor("bandh1", [128, 128], DT.bfloat16,
                              kind="ExternalInput")
    rowind4_d = nc.dram_tensor("rowind4", [4, 128], DT.bfloat16,
                               kind="ExternalInput")
    ident_d = nc.dram_tensor("identb", [128, 128], DT.bfloat16,
                             kind="ExternalInput")


    with _TileContext(nc) as tc, contextlib.ExitStack() as ctx:
        cpool = ctx.enter_context(tc.tile_pool(name="consts", bufs=1))
        xpool = ctx.enter_context(tc.tile_pool(name="xp", bufs=2))
        xrpool = ctx.enter_context(tc.tile_pool(name="xrp", bufs=3))
        sqp = ctx.enter_context(tc.tile_pool(name="sqp", bufs=2))
        qkp = ctx.enter_context(tc.tile_pool(name="qkp", bufs=6))
        vvp = ctx.enter_context(tc.tile_pool(name="vvp", bufs=2))
        vpp = ctx.enter_context(tc.tile_pool(name="vpp", bufs=2))
        phap = ctx.enter_context(tc.tile_pool(name="phap", bufs=3))
        ostp = ctx.enter_context(tc.tile_pool(name="ostp", bufs=3))
        ohp = ctx.enter_context(tc.tile_pool(name="ohp", bufs=2))
        o1p = ctx.enter_context(tc.tile_pool(name="o1p", bufs=2))
        xhp = ctx.enter_context(tc.tile_pool(name="xhp", bufs=2))
        gelp = ctx.enter_context(tc.tile_pool(name="gelp", bufs=4))
        o2p = ctx.enter_context(tc.tile_pool(name="o2p", bufs=2))
        rows = ctx.enter_context(tc.tile_pool(name="rows", bufs=3))
        dense = ctx.enter_context(tc.tile_pool(name="dense", bufs=1))
        ps = ctx.enter_context(tc.tile_pool(name="ps", bufs=4, space="PSUM"))

        # ---- constants ----
        def cload(name, shape, dt, dram):
            t = cpool.tile(shape, dt, tag=name, name=name)
            nc.sync.dma_start(t[:], dram[:])
            return t

        wqkv_sb = cload("wqkv", [128, 2, 3 * C], DT.bfloat16, wqkv2_d)
        corr2_sb = cload("corr2", [2, 3 * C], DT.bfloat16, corr2_d)
        tapw = cload("tapw", [128, 6, 2], DT.float32, tapw_d)
        wproj_sb = cload("wproj", [128, 2, C], DT.bfloat16, wproj_d)
        w1_sb = cload("w1", [128, 2, HID], DT.bfloat16, w1_d)
        w1cs_sb = cload("w1cs", [1, HID], DT.bfloat16, w1cs_d)
        w2_sb = cload("w2", [128, 4, C], DT.bfloat16, w2_d)
        b1v = cload("b1v", [128, 4], DT.float32, b1v_d)
        b2v = cload("b2v", [128, 2], DT.float32, b2v_d)
        temp128 = cload("temp", [128, 1], DT.float32, temp_d)
        onesb = cload("onesb", [128, 128], DT.bfloat16, onesb_d)
        bandh = [cload("bandh0", [128, 128], DT.bfloat16, bandh0_d),
                 cload("bandh1", [128, 128], DT.bfloat16, bandh1_d)]
        rowind4 = cload("rowind4", [4, 128], DT.bfloat16, rowind4_d)
        identb = cload("identb", [128, 128], DT.bfloat16, ident_d)

        # xn: static pair, plain layout (conv guards live in qpre now)
        xn_st = [[cpool.tile([128, W], DT.bfloat16,
                             name=f"xn{i}_{kt}", tag=f"xn{i}_{kt}")
                  for kt in range(2)] for i in range(1)]
        # qpre: guarded staging for the depthwise conv (zero guard cols once)
        qpre_st = [cpool.tile([128, S, PX], DT.bfloat16,
                              name=f"qpre{i}", tag=f"qpre{i}")
                   for i in range(3)]
        for i in range(3):
            nc.vector.memset(qpre_st[i][:, :, 0:1], 0.0)
            nc.vector.memset(qpre_st[i][:, :, PX - 1:PX], 0.0)

        # ---- dense helpers (gpsimd newton chains, [*,128] tiles) ----
        def rsqrt_dense(x_ap, out_ap, iters=2, eng=None):
            eng = eng or nc.gpsimd
            shape = list(x_ap.shape)
            s1 = dense.tile(shape, DT.int32, tag="nw_i1", name="nw_i1")
            nc.vector.tensor_scalar(s1[:], x_ap.bitcast(DT.int32), 1, None,
                                    ALU.arith_shift_right)
            nc.vector.tensor_scalar(s1[:], s1[:], -1, None, ALU.bitwise_xor)
            nc.vector.tensor_scalar(s1[:], s1[:], RSQRT_MAGIC + 1, None,
                                    ALU.add)
            y = s1[:].bitcast(DT.float32)
            for it in range(iters):
                t = dense.tile(shape, DT.float32, tag="nw_t", name="nw_t")
                u = dense.tile(shape, DT.float32, tag="nw_u", name="nw_u")
                eng.tensor_mul(t[:], y, y)
                eng.tensor_scalar(t[:], t[:], -0.5, None, ALU.mult)
                eng.tensor_mul(u[:], t[:], x_ap)
                eng.tensor_scalar(u[:], u[:], 1.5, None, ALU.add)
                last = (it == iters - 1)
                ynew = out_ap if last else dense.tile(
                    shape, DT.float32, name="nw_y", tag="nw_y", bufs=2)
                yap = ynew if last else ynew[:]
                eng.tensor_mul(yap, u[:], y)
                y = yap

        def recip_dense(x_ap, out_ap, iters=2, eng=None):
            eng = eng or nc.gpsimd
            shape = list(x_ap.shape)
            s1 = dense.tile(shape, DT.int32, tag="nw_i1", name="nw_i1")
            nc.vector.tensor_scalar(s1[:], x_ap.bitcast(DT.int32), -1, None,
                                    ALU.bitwise_xor)
            nc.vector.tensor_scalar(s1[:], s1[:], 0x7EF127EA + 1, None,
                                    ALU.add)
            y = s1[:].bitcast(DT.float32)
            for it in range(iters):
                u = dense.tile(shape, DT.float32, tag="nw_t", name="nw_t")
                eng.tensor_mul(u[:], x_ap, y)
                v = dense.tile(shape, DT.float32, tag="nw_u", name="nw_u")
                eng.tensor_scalar(v[:], u[:], -1.0, None, ALU.mult)
                eng.tensor_scalar(v[:], v[:], 2.0, None, ALU.add)
                last = (it == iters - 1)
                ynew = out_ap if last else dense.tile(
                    shape, DT.float32, name="nw_y", tag="nw_y", bufs=2)
                yap = ynew if last else ynew[:]
                eng.tensor_mul(yap, v[:], y)
                y = yap

        st = {b: {} for b in range(NBLK)}

        # ---------------- stages ----------------
        def s0_load(blk):
            d = st[blk]
            d["xb"] = [sqp.tile([128, W], DT.bfloat16, name=f"xb_{kt}",
                                tag="xb") for kt in range(2)]
            for kt in range(2):
                for h2 in range(2):
                    xh32 = xpool.tile([128, S // 2, Fd], DT.float32,
                                      name="xh32", tag="x32")
                    nc.sync.dma_start(
                        xh32[:],
                        x_in[kt * 128:(kt + 1) * 128,
                             blk * S + h2 * 4:blk * S + (h2 + 1) * 4, :])
                    nc.vector.tensor_copy(
                        d["xb"][kt][:, h2 * HW_:(h2 + 1) * HW_],
                        xh32[:].rearrange("p s f -> p (s f)"))

        def _stats(blk, srcs, sqs, which, fr):
            """Partition-sum stats of srcs (and sqs) -> dense mu*rsig and
            rsig rows (bf16 [16,128]) stored as mr{which}/rr{which}."""
            d = st[blk]
            su_row = rows.tile([1, W], DT.bfloat16,
                               name=f"su_{which}", tag="rows")
            sq_row = rows.tile([1, W], DT.bfloat16,
                               name=f"sqr_{which}", tag="rows")
            for h2 in range(2):
                ps_su = ps.tile([1, HW_], DT.float32, tag="mm", name="ps_su")
                ps_sq = ps.tile([1, HW_], DT.float32, tag="mm", name="ps_sq")
                for ch in range(2):
                    sl = slice(h2 * HW_ + ch * 512, h2 * HW_ + (ch + 1) * 512)
                    psl = slice(ch * 512, (ch + 1) * 512)
                    for kt in range(2):
                        nc.tensor.matmul(
                            ps_su[0:1, psl], onesb[:, 0:1],
                            srcs[kt][:, sl],
                            start=(kt == 0), stop=(kt == 1),
                            skip_group_check=True)
                    for kt in range(2):
                        nc.tensor.matmul(
                            ps_sq[0:1, psl], onesb[:, 0:1],
                            sqs[(kt, h2)][:, psl],
                            start=(kt == 0), stop=(kt == 1),
                            skip_group_check=True)
                nc.scalar.activation(su_row[0:1, h2 * HW_:(h2 + 1) * HW_],
                                     ps_su[:], AF.Copy)
                nc.scalar.activation(sq_row[0:1, h2 * HW_:(h2 + 1) * HW_],
                                     ps_sq[:], AF.Copy)
            dsu = dense.tile([16, 128], DT.bfloat16, tag="dsu", name="dsu")
            dsq = dense.tile([16, 128], DT.bfloat16, tag="dsq", name="dsq")
            nc.sync.dma_start(
                dsu[:], su_row[:].rearrange("o (j c) -> o j c", c=128))
            nc.sync.dma_start(
                dsq[:], sq_row[:].rearrange("o (j c) -> o j c", c=128))
            mu = dense.tile([16, 128], DT.float32, tag="dmu", name="dmu")
            nc.vector.tensor_scalar(mu[:], dsu[:], 1.0 / C, None, ALU.mult)
            var = dense.tile([16, 128], DT.float32, tag="dvar", name="dvar")
            m2 = dense.tile([16, 128], DT.float32, tag="nw_t", name="dm2")
            nc.vector.tensor_mul(m2[:], mu[:], mu[:])
            nc.vector.tensor_scalar(var[:], dsq[:], 1.0 / C, LN_EPS,
                                    ALU.mult, ALU.add)
            nc.vector.tensor_sub(var[:], var[:], m2[:])
            rsd = dense.tile([16, 128], DT.float32, tag="drs", name="drs")
            rsqrt_dense(var[:], rsd[:], iters=1, eng=nc.vector)
            mr = dense.tile([16, 128], DT.bfloat16, tag="dmr", name="dmr")
            nc.vector.tensor_mul(mr[:], mu[:], rsd[:])
            rr = dense.tile([16, 128], DT.bfloat16, tag="drr", name="drr")
            nc.vector.tensor_copy(rr[:], rsd[:])
            d[f"mr{which}"] = mr
            d[f"rr{which}"] = rr

        def s1_ln1_stats(blk):
            d = st[blk]
            srcs = [d["xb"][kt][:] for kt in range(2)]
            sqs = {}
            for kt in range(2):
                for h2 in range(2):
                    x2 = sqp.tile([128, HW_], DT.bfloat16, name="x2",
                                  tag="sqh")
                    s_ = srcs[kt][:, h2 * HW_:(h2 + 1) * HW_]
                    nc.vector.scalar_tensor_tensor(
                        x2[:], s_, 1.0, s_, ALU.mult, ALU.mult)
                    sqs[(kt, h2)] = x2
            _stats(blk, srcs, sqs, 1, fr=False)

        def s2_ln1_apply(blk):
            d = st[blk]
            mr, rr = d.pop("mr1"), d.pop("rr1")
            rrow = rows.tile([1, W], DT.bfloat16, name="rrow", tag="rows")
            nc.sync.dma_start(
                rrow[:].rearrange("o (j c) -> o j c", c=128), rr[:])
            corrR = rows.tile([2, W], DT.bfloat16, name="corrR", tag="rows")
            nc.sync.dma_start(
                corrR[0:1, :].rearrange("o (j c) -> o j c", c=128), mr[:])
            nc.sync.dma_start(corrR[1:2, :], corrR_d[1:2, :])
            d["corrR"] = corrR
            xn16 = xn_st[0]
            for h2 in range(2):
                ps_r = ps.tile([128, HW_], DT.float32, tag="mm", name="ps_r")
                for ch in range(2):
                    nc.tensor.matmul(
                        ps_r[:, ch * 512:(ch + 1) * 512],
                        onesb[0:1, :],
                        rrow[0:1, h2 * HW_ + ch * 512:
                             h2 * HW_ + (ch + 1) * 512],
                        start=True, stop=True, skip_group_check=True)
                for kt in range(2):
                    nc.vector.tensor_mul(
                        xn16[kt][:, h2 * HW_:(h2 + 1) * HW_],
                        d["xb"][kt][:, h2 * HW_:(h2 + 1) * HW_],
                        ps_r[:])
            d["xn"] = xn16

        def qkv_m(blk, m, qc):
            d = st[blk]
            qp3 = qpre_st[m % 3][:]
            if m < 4:
                qt = qkp.tile([128, W], DT.bfloat16, name=f"qc{m}", tag="qk")
            else:
                qt = vvp.tile([128, W], DT.bfloat16, name=f"vc{m}", tag="vv")
            qc.append(qt)
            for h2 in range(2):
                ps_m = ps.tile([128, HW_], DT.float32, tag="mm", name="ps_m")
                for ch in range(2):
                    sl = slice(h2 * HW_ + ch * 512, h2 * HW_ + (ch + 1) * 512)
                    for kt in range(2):
                        nc.tensor.matmul(
                            ps_m[:, ch * 512:(ch + 1) * 512],
                            wqkv_sb[:, kt, m * 128:(m + 1) * 128],
                            d["xn"][kt][:, sl],
                            start=(kt == 0), stop=False,
                            skip_group_check=True)
                    nc.tensor.matmul(
                        ps_m[:, ch * 512:(ch + 1) * 512],
                        corr2_sb[:, m * 128:(m + 1) * 128],
                        d["corrR"][:, sl],
                        start=False, stop=True, skip_group_check=True)
                nc.vector.tensor_copy(
                    qp3[:, h2 * 4:(h2 + 1) * 4, 1:1 + Fd],
                    ps_m[:].rearrange("p (s f) -> p s f", f=Fd))
            # depthwise conv3 along f (middle tap folded into Wqkv):
            # qc = qpre + r0*shift(-1) + r2*shift(+1), in place (bf16 DVE)
            vm1 = qp3[:, :, 0:Fd]
            v00 = qp3[:, :, 1:1 + Fd]
            vp1 = qp3[:, :, 2:2 + Fd]
            qf = qc[m][:].rearrange("p (s f) -> p s f", f=Fd)
            nc.vector.scalar_tensor_tensor(qf, vm1, tapw[:, m, 0:1], v00,
                                           ALU.mult, ALU.add)
            nc.vector.scalar_tensor_tensor(qf, vp1, tapw[:, m, 1:2], qf,
                                           ALU.mult, ALU.add)

        def qkv_tail(blk, qc):
            d = st[blk]
            d.pop("corrR")
            d["qc"] = qc[:4]
            # v transpose (frees v tiles fast)
            vp = [vpp.tile([128, S, H, Dh + 1], DT.bfloat16,
                           name=f"vp{b2}", tag=f"vp{b2}") for b2 in range(2)]
            for b2 in range(2):
                nc.vector.memset(vp[b2][:, :, :, Dh:Dh + 1], 1.0)
            for ti in range(2):
                vt = qc[4 + ti]
                for b2 in range(2):
                    ps_tp = ps.tile([128, S * 128], DT.bfloat16, tag="mm",
                                    name="ps_tp")
                    for s in range(S):
                        nc.tensor.transpose(
                            ps_tp[:, s * 128:(s + 1) * 128],
                            vt[:, s * Fd + b2 * 128:s * Fd + b2 * 128 + 128],
                            identb[:])
                    nc.vector.tensor_copy(
                        vp[b2][:, :, 4 * ti:4 * ti + 4, 0:Dh],
                        ps_tp[:].rearrange("p (s hb d) -> p s hb d",
                                           s=S, hb=4))
            d["vp"] = vp

        def s3_qkv(blk):
            qc = []
            for m in range(6):
                qkv_m(blk, m, qc)
            qkv_tail(blk, qc)

        def s4_l2sumsq(blk):
            d = st[blk]
            d["dnq"] = {}
            for vi, base in (("q", 0), ("k", 2)):
                d_n = dense.tile([128, 128], DT.bfloat16, tag="dn", name="dn")
                for h2 in range(2):
                    ps_n = ps.tile([128, HW_], DT.float32, tag="mm",
                                   name="ps_n")
                    for ti in range(2):
                        sq = sqp.tile([128, HW_], DT.bfloat16, name="l2sq",
                                      tag="sqh")
                        qs = d["qc"][base + ti][:, h2 * HW_:(h2 + 1) * HW_]
                        nc.scalar.activation(sq[:], qs, AF.Square)
                        for ch in range(2):
                            nc.tensor.matmul(
                                ps_n[:, ch * 512:(ch + 1) * 512],
                                bandh[ti][:], sq[:, ch * 512:(ch + 1) * 512],
                                start=(ti == 0), stop=(ti == 1),
                                skip_group_check=True)
                    nsb = rows.tile([8, HW_], DT.bfloat16, name="nsb",
                                    tag="rows")
                    nc.any.tensor_copy(nsb[:], ps_n[0:8, :])
                    nc.sync.dma_start(
                        d_n[h2 * 64:(h2 + 1) * 64, :],
                        nsb[:].rearrange("h (j c) -> h j c", c=128))
                dnf = dense.tile([128, 128], DT.float32, tag="dnf",
                                 name="dnf")
                nc.vector.tensor_copy(dnf[:], d_n[:])
                r_n = dense.tile([128, 128], DT.float32, tag="dr", name="dr")
                rsqrt_dense(dnf[:], r_n[:], iters=1, eng=nc.vector)
                r16 = dense.tile([128, 128], DT.bfloat16, tag="dr16",
                                 name="dr16")
                if vi == "k":
                    nc.vector.tensor_scalar(r16[:], r_n[:], temp128[:, 0:1],
                                            None, ALU.mult)
                else:
                    nc.vector.tensor_copy(r16[:], r_n[:])
                d["dnq"][vi] = r16

        def s5_l2apply(blk):
            d = st[blk]
            for vi, base in (("q", 0), ("k", 2)):
                r16 = d["dnq"].pop(vi)
                for ti in range(2):
                    r4 = rows.tile([4, W], DT.bfloat16, name="r4", tag="rows")
                    for h2 in range(2):
                        nc.sync.dma_start(
                            r4[:, h2 * HW_:(h2 + 1) * HW_].rearrange(
                                "b (j c) -> b j c", c=128),
                            r16[h2 * 64 + ti * 32:h2 * 64 + ti * 32 + 32, :])
                    for h2 in range(2):
                        ps_b = ps.tile([128, HW_], DT.float32, tag="mm",
                                       name="ps_b")
                        for ch in range(2):
                            sl = slice(h2 * HW_ + ch * 512,
                                       h2 * HW_ + (ch + 1) * 512)
                            nc.tensor.matmul(
                                ps_b[:, ch * 512:(ch + 1) * 512],
                                rowind4[:], r4[:, sl],
                                start=True, stop=True, skip_group_check=True)
                        qs = d["qc"][base + ti][:, h2 * HW_:(h2 + 1) * HW_]
                        nc.vector.tensor_mul(qs, qs, ps_b[:])
            d.pop("dnq")

        def attn_h(blk, h, oh16, d_rs):
            d = st[blk]
            ti, band = h // 4, (h % 4) * 32
            phat = []
            for jt in range(2):
                pj = phap.tile([128, W], DT.bfloat16, tag="phat",
                               name="phat")
                for h2 in range(2):
                    ps_S = ps.tile([128, HW_], DT.float32, tag="mm",
                                   name="ps_S")
                    for si in range(4):
                        s = h2 * 4 + si
                        nc.tensor.matmul(
                            ps_S[:, si * Fd:(si + 1) * Fd],
                            d["qc"][2 + ti][
                                band:band + 32,
                                s * Fd + jt * 128:s * Fd + jt * 128 + 128],
                            d["qc"][ti][band:band + 32,
                                        s * Fd:(s + 1) * Fd],
                            start=True, stop=True, skip_group_check=True,
                            tile_position=(band, 0))
                    nc.scalar.activation(
                        pj[:, h2 * HW_:(h2 + 1) * HW_], ps_S[:], AF.Exp)
                phat.append(pj)
            ostg = ostp.tile([Dh + 1, W], DT.bfloat16, tag="ostg",
                             name="ostg")
            for h2 in range(2):
                ps_O = ps.tile([Dh + 1, HW_], DT.float32, tag="mm",
                               name="ps_O")
                for si in range(4):
                    s = h2 * 4 + si
                    for ktj in range(2):
                        nc.tensor.matmul(
                            ps_O[:, si * Fd:(si + 1) * Fd],
                            d["vp"][ktj][:, s, h, :],
                            phat[ktj][:, s * Fd:(s + 1) * Fd],
                            start=(ktj == 0), stop=(ktj == 1),
                            skip_group_check=True)
                nc.any.tensor_copy(ostg[:, h2 * HW_:(h2 + 1) * HW_],
                                   ps_O[:])
            nc.gpsimd.dma_start(oh16[ti][band:band + 32, :],
                                ostg[0:Dh, :])
            nc.sync.dma_start(
                d_rs[ti][(h % 4) * 16:(h % 4 + 1) * 16, :],
                ostg[Dh:Dh + 1, :].rearrange("o (j c) -> o j c", c=128))

        def attn_head_setup(blk):
            d = st[blk]
            oh16 = [ohp.tile([128, W], DT.bfloat16, name=f"oh{ti}", tag="oh")
                    for ti in range(2)]
            d_rs = [dense.tile([64, 128], DT.bfloat16, tag=f"dnr{ti}",
                               name=f"dnr{ti}") for ti in range(2)]
            d["oh16"] = oh16
            return oh16, d_rs

        def attn_tail_ti(blk, ti, d_rs):
            d = st[blk]
            drf = dense.tile([64, 128], DT.float32, tag="drf", name="drf",
                             bufs=2)
            nc.vector.tensor_copy(drf[:], d_rs[ti][:])
            d_ri = dense.tile([64, 128], DT.bfloat16, tag=f"dri{ti}",
                              name=f"dri{ti}")
            recip_dense(drf[:], d_ri[:], eng=nc.vector)
            d[f"d_ri{ti}"] = d_ri

        def s6_attn(blk):
            oh16, d_rs = attn_head_setup(blk)
            for h in range(H):
                attn_h(blk, h, oh16, d_rs)
                if h == 3:
                    attn_tail_ti(blk, 0, d_rs)
            attn_tail_ti(blk, 1, d_rs)

        def s7_denom(blk, ti):
            d = st[blk]
            d_ri = d.pop(f"d_ri{ti}")
            r4 = rows.tile([4, W], DT.bfloat16, name="rinv", tag="rows")
            nc.sync.dma_start(
                r4[:].rearrange("b (j c) -> b j c", c=128),
                d_ri[:])
            for h2 in range(2):
                ps_b = ps.tile([128, HW_], DT.float32, tag="mm",
                               name="ps_b2")
                for ch in range(2):
                    sl = slice(h2 * HW_ + ch * 512,
                               h2 * HW_ + (ch + 1) * 512)
                    nc.tensor.matmul(
                        ps_b[:, ch * 512:(ch + 1) * 512],
                        rowind4[:], r4[:, sl],
                        start=True, stop=True, skip_group_check=True)
                ohs = d["oh16"][ti][:, h2 * HW_:(h2 + 1) * HW_]
                nc.vector.tensor_mul(ohs, ohs, ps_b[:])

        def s7_proj(blk):
            d = st[blk]
            d["xr"] = {}
            for m2 in range(2):
                for h2 in range(2):
                    xr32 = xrpool.tile([128, S // 2, Fd], DT.float32,
                                       name="xr32", tag="xr32")
                    nc.sync.dma_start(
                        xr32[:],
                        x_in[m2 * 128:(m2 + 1) * 128,
                             blk * S + h2 * 4:blk * S + (h2 + 1) * 4, :])
                    d["xr"][(m2, h2)] = xr32
            for ti in range(2):
                if f"d_ri{ti}" in d:
                    s7_denom(blk, ti)
            o1 = [o1p.tile([128, W], DT.bfloat16, name=f"o1_{m}", tag="o1")
                  for m in range(2)]
            for m2 in range(2):
                for h2 in range(2):
                    xr32 = xrpool.tile([128, S // 2, Fd], DT.float32,
                                       name="xr32", tag="xr32")
                    nc.sync.dma_start(
                        xr32[:],
                        x_in[m2 * 128:(m2 + 1) * 128,
                             blk * S + h2 * 4:blk * S + (h2 + 1) * 4, :])
                    xf = xr32[:].rearrange("p s f -> p (s f)")
                    ps_y = ps.tile([128, HW_], DT.float32, tag="mm",
                                   name="ps_y")
                    for ch in range(2):
                        sl = slice(h2 * HW_ + ch * 512,
                                   h2 * HW_ + (ch + 1) * 512)
                        for kt in range(2):
                            nc.tensor.matmul(
                                ps_y[:, ch * 512:(ch + 1) * 512],
                                wproj_sb[:, kt, m2 * 128:(m2 + 1) * 128],
                                d["oh16"][kt][:, sl],
                                start=(kt == 0), stop=(kt == 1),
                                skip_group_check=True)
                    nc.vector.scalar_tensor_tensor(
                        o1[m2][:, h2 * HW_:(h2 + 1) * HW_],
                        ps_y[:], 1.0, xf[:], ALU.mult, ALU.add)
            d["o1"] = o1
            d.pop("oh16")
            d.pop("qc")
            d.pop("vp")
            d.pop("xn")
            d.pop("xb")

        def s8_ln2_stats(blk):
            d = st[blk]
            srcs = [d["o1"][kt][:] for kt in range(2)]
            sqs = {}
            for kt in range(2):
                for h2 in range(2):
                    x2 = sqp.tile([128, HW_], DT.bfloat16, name="o1sq",
                                  tag="sqh")
                    s_ = srcs[kt][:, h2 * HW_:(h2 + 1) * HW_]
                    nc.scalar.activation(x2[:], s_, AF.Square)
                    sqs[(kt, h2)] = x2
            _stats(blk, srcs, sqs, 2, fr=False)

        def mlp_head(blk):
            d = st[blk]
            mr, rr = d.pop("mr2"), d.pop("rr2")
            rrow = rows.tile([1, W], DT.bfloat16, name="rrow2", tag="rows")
            nc.sync.dma_start(
                rrow[:].rearrange("o (j c) -> o j c", c=128), rr[:])
            mrow = rows.tile([1, W], DT.bfloat16, name="mrow2", tag="rows")
            nc.sync.dma_start(
                mrow[:].rearrange("o (j c) -> o j c", c=128), mr[:])
            xh = [xhp.tile([128, W], DT.bfloat16, name=f"xh{kt}", tag="xh")
                  for kt in range(2)]
            for h2 in range(2):
                ps_r = ps.tile([128, HW_], DT.float32, tag="mm", name="ps_r2")
                for ch in range(2):
                    nc.tensor.matmul(
                        ps_r[:, ch * 512:(ch + 1) * 512],
                        onesb[0:1, :],
                        rrow[0:1, h2 * HW_ + ch * 512:
                             h2 * HW_ + (ch + 1) * 512],
                        start=True, stop=True, skip_group_check=True)
                for kt in range(2):
                    nc.vector.tensor_mul(
                        xh[kt][:, h2 * HW_:(h2 + 1) * HW_],
                        d["o1"][kt][:, h2 * HW_:(h2 + 1) * HW_],
                        ps_r[:])
            g16 = [gelp.tile([128, W], DT.bfloat16, name=f"gel{m}", tag="gel")
                   for m in range(4)]
            d["xh"] = xh
            d["mrow"] = mrow
            d["g16"] = g16

        def mlp_w1(blk, mh):
            d = st[blk]
            xh, mrow, g16 = d["xh"], d["mrow"], d["g16"]
            for h2 in range(2):
                ps_h = ps.tile([128, HW_], DT.float32, tag="mm",
                               name="ps_h")
                for ch in range(2):
                    sl = slice(h2 * HW_ + ch * 512,
                               h2 * HW_ + (ch + 1) * 512)
                    for kt in range(2):
                        nc.tensor.matmul(
                            ps_h[:, ch * 512:(ch + 1) * 512],
                            w1_sb[:, kt, mh * 128:(mh + 1) * 128],
                            xh[kt][:, sl],
                            start=(kt == 0), stop=False,
                            skip_group_check=True)
                    nc.tensor.matmul(
                        ps_h[:, ch * 512:(ch + 1) * 512],
                        w1cs_sb[0:1, mh * 128:(mh + 1) * 128],
                        mrow[0:1, sl],
                        start=False, stop=True, skip_group_check=True)
                nc.scalar.activation(
                    g16[mh][:, h2 * HW_:(h2 + 1) * HW_], ps_h[:],
                    AF.Gelu, bias=b1v[:, mh:mh + 1], scale=1.0)

        def mlp_w2(blk, m2, h2):
            d = st[blk]
            g16 = d["g16"]
            ps_y = ps.tile([128, HW_], DT.float32, tag="mm", name="ps_y2")
            for ch in range(2):
                for kt in range(4):
                    nc.tensor.matmul(
                        ps_y[:, ch * 512:(ch + 1) * 512],
                        w2_sb[:, kt, m2 * 128:(m2 + 1) * 128],
                        g16[kt][:, h2 * HW_ + ch * 512:
                                h2 * HW_ + (ch + 1) * 512],
                        start=(kt == 0), stop=(kt == 3),
                        skip_group_check=True)
            o2 = o2p.tile([128, HW_], DT.float32, tag="o2", name="o2t")
            nc.vector.scalar_tensor_tensor(
                o2[:], ps_y[:], b2v[:, m2:m2 + 1],
                d["o1"][m2][:, h2 * HW_:(h2 + 1) * HW_],
                ALU.add, ALU.add)
            nc.sync.dma_start(
                out_d[m2 * 128:(m2 + 1) * 128,
                      blk * S + h2 * 4:blk * S + (h2 + 1) * 4, :],
                o2[:].rearrange("p (s f) -> p s f", f=Fd))

        def mlp_tail(blk):
            d = st[blk]
            d.pop("xh")
            d.pop("mrow")
            d.pop("g16")
            d.pop("o1")

        def s9_mlp(blk):
            mlp_head(blk)
            for mh in range(4):
                mlp_w1(blk, mh)
            for m2 in range(2):
                for h2 in range(2):
                    mlp_w2(blk, m2, h2)
            mlp_tail(blk)

        def merged_step(bm, ba, bq):
            """Interleave mlp(bm), attention(ba), qkv(bq); any may be None."""
            if ba is not None:
                oh16, d_rs = attn_head_setup(ba)
            if bm is not None:
                mlp_head(bm)
            qc = []
            plan = []
            for i in range(8):
                if ba is not None:
                    plan.append(("h", i))
                if bm is not None and i < 4:
                    plan.append(("w1", i))
                if bm is not None and 4 <= i < 8:
                    plan.append(("w2", i - 4))
                if bq is not None and 1 <= i < 7:
                    plan.append(("m", i - 1))
            for kind, i in plan:
                if kind == "h":
                    attn_h(ba, i, oh16, d_rs)
                    if i == 3:
                        attn_tail_ti(ba, 0, d_rs)
                    elif i == 7:
                        attn_tail_ti(ba, 1, d_rs)
                elif kind == "w1":
                    mlp_w1(bm, i)
                elif kind == "w2":
                    mlp_w2(bm, i // 2, i % 2)
                else:
                    qkv_m(bq, i, qc)
            if bq is not None:
                qkv_tail(bq, qc)
            if bm is not None:
                mlp_tail(bm)

        stages = [s0_load, s1_ln1_stats, s2_ln1_apply, s3_qkv, s4_l2sumsq,
                  s5_l2apply, s6_attn, s7_proj, s8_ln2_stats, s9_mlp]


        # skew-3 wavefront: later stages (lower block index) first.
        # s9(b-1), s6(b), s3(b+1) land on the same step; emit them
        # interleaved so the shared psum ring rotates through all three.
        nst = len(stages)
        for t in range(nst + SKEW * (NBLK - 1)):
            todo = [(b, t - SKEW * b) for b in range(NBLK)
                    if 0 <= t - SKEW * b < nst]
            jmap = {j: b for (b, j) in todo}
            skip = set()
            if 6 in jmap:
                bm = jmap.get(9)
                ba = jmap[6]
                bq = jmap.get(3)
                for j in (9, 6, 3):
                    if j in jmap:
                        skip.add((jmap[j], j))
            order = sorted(todo, key=lambda bj: (0 if bj[1] == 4 else 1,
                                                 bj[0]))
            for b, j in order:
                if (b, j) in skip:
                    if j == 6:
                        merged_step(jmap.get(9), b, jmap.get(3))
                    continue
                stages[j](b)

    _split_excess_waits(nc, max_waits=1)
    return nc


def _host_prep(inputs):
    Wqkv = np.asarray(inputs["Wqkv"], np.float64)        # (C, 3C)
    g1 = np.asarray(inputs["norm1_g"], np.float64)
    b1n = np.asarray(inputs["norm1_b"], np.float64)
    g2 = np.asarray(inputs["norm2_g"], np.float64)
    b2n = np.asarray(inputs["norm2_b"], np.float64)
    dw_w = np.asarray(inputs["dw_w"], np.float64)
    taps = dw_w[:, 0, :]                                 # (3C, 3)

    Wq = Wqkv * g1[:, None]                              # g1 folded
    # fold the middle conv tap into the weights; conv uses tap ratios
    w1t = taps[:, 1].copy()
    w1t = np.where(np.abs(w1t) < 1e-30, 1e-30, w1t)
    Wqf = Wq * w1t[None, :]
    wqkv2 = np.ascontiguousarray(
        Wqf.reshape(2, 128, 3 * C).transpose(1, 0, 2)).astype(BF16)

    colsum = Wqf.sum(axis=0)                             # (3C,)
    bq = (b1n @ Wqkv) * w1t                              # (3C,)
    corr2 = np.stack([-colsum, bq]).astype(BF16)         # (2, 3C)
    corrR = np.zeros((2, S * Fd), np.float32)
    corrR[1] = 1.0                                       # static ones row
    ratios = np.stack([taps[:, 0] / w1t, taps[:, 2] / w1t], axis=1)
    tapw = np.ascontiguousarray(
        ratios.reshape(6, 128, 2).transpose(1, 0, 2)).astype(np.float32)

    def kt_major(w, nkt):
        K, N = w.shape
        return np.ascontiguousarray(
            w.reshape(nkt, 128, N).transpose(1, 0, 2)).astype(BF16)

    wproj = kt_major(np.asarray(inputs["Wproj"], np.float64), 2)
    W1 = np.asarray(inputs["W1"], np.float64)
    W1g = W1 * g2[:, None]
    w1 = kt_major(W1g, 2)
    w1cs = (-W1g.sum(axis=0)).reshape(1, HID).astype(BF16)
    w2 = kt_major(np.asarray(inputs["W2"], np.float64), 4)

    b1p = np.asarray(inputs["b1"], np.float64) + b2n @ W1
    b1v = np.ascontiguousarray(b1p.reshape(4, 128).T).astype(np.float32)
    b2v = np.ascontiguousarray(
        np.asarray(inputs["b2"], np.float32).reshape(2, 128).T)

    temp = np.asarray(inputs["temperature"], np.float32).reshape(H)
    # l2 dense rows are h2-major: row = h2*64 + head*8 + j
    temp128 = np.array([temp[(r % 64) // 8] for r in range(128)],
                       np.float32).reshape(128, 1)

    bandh0 = np.zeros((128, 128), np.float32)
    bandh1 = np.zeros((128, 128), np.float32)
    for dd in range(128):
        for m in range(128):
            if m % 8 == dd // 32:
                bandh0[dd, m] = 1.0
            if m % 8 == 4 + dd // 32:
                bandh1[dd, m] = 1.0
    rowind4 = np.zeros((4, 128), np.float32)
    for m in range(128):
        rowind4[m // 32, m] = 1.0

    return dict(
        wqkv2=wqkv2, corr2=corr2, corrR=corrR.astype(BF16), tapw=tapw,
        wproj=wproj, w1=w1, w1cs=w1cs, w2=w2,
        b1v=b1v, b2v=b2v, temp128=temp128,
        onesb=np.ones((128, 128), BF16),
        bandh0=bandh0.astype(BF16),
        bandh1=bandh1.astype(BF16),
        rowind4=rowind4.astype(BF16),
        identb=np.eye(128).astype(BF16),
    )


_NC_CACHE = {}


def get_nc():
    if "nc" not in _NC_CACHE:
        _NC_CACHE["nc"] = build_nc()
    return _NC_CACHE["nc"]


def make_in_maps(inputs):
    consts = _host_prep(inputs)
    x = np.asarray(inputs["x"], np.float32)  # (B, C, T, Fd)
    in_maps = []
    for core in range(NCORES):
        b, t0 = core // 2, (core % 2) * SPC
        m = dict(consts)
        m["x"] = np.ascontiguousarray(x[b, :, t0:t0 + SPC, :])
        in_maps.append(m)
    return in_maps


def assemble_out(results):
    out = np.zeros((B, C, T, Fd), np.float32)
    for core in range(NCORES):
        b, t0 = core // 2, (core % 2) * SPC
        out[b, :, t0:t0 + SPC, :] = results[core]["out"]
    return out


def kernel(**inputs):
    nc = get_nc()
    in_maps = make_in_maps(inputs)
    res = run_bass_kernel_spmd(nc, in_maps, core_ids=list(range(NCORES)))
    return assemble_out(res.results)


# revision 9
# speedup vs baseline: 1.0612x; 1.0144x over previous
"""Trainium2 Bass kernel v2 for the AxisMDTA dense-transformer block.

x (4, 256, 64, 256) fp32 -> out (4, 256, 64, 256) fp32.
Data-parallel over the 256 (b,t) samples across 8 NeuronCores (32/core).
Channel-major on-chip layout (c on partitions, (sample, f) on free dim).

v2 vs baseline:
- LN gammas folded into following weights host-side; LN bias + mean
  subtraction folded into the qkv / W1 matmuls as rank-k correction
  matmuls (shifted guarded rows handle the depthwise-conv taps exactly).
- Stats matmuls run f32r directly on fp32 x (no bf16 staging copy);
  squares via scalar_tensor_tensor (DVE 2x modes); dense Newton chains
  on gpsimd.
- Uniform [128,1024] PSUM quanta from one 4-deep ring.
- Stage emission is a skew-3 software-pipeline wavefront across the 4
  blocks so the PE stream always has ready work behind a stalled op.
- Guard columns zeroed once; per-block memsets eliminated.
- bf16 trunk (o1); x reloaded from DRAM for the proj residual; band
  moves via SWDGE (gpsimd) to offload HWDGE.
"""

import contextlib

import numpy as np
import ml_dtypes

import concourse.bass as bass
import concourse.mybir as mybir
import concourse.tile as tile
from concourse.vector_clock import ScopedClock
from concourse.bass_utils import run_bass_kernel_spmd

AF = mybir.ActivationFunctionType
ALU = mybir.AluOpType
DT = mybir.dt
BF16 = ml_dtypes.bfloat16

B, C, T, Fd = 4, 256, 64, 256
H, Dh = 8, 32
HID = 512
NCORES = 8
SPC = (B * T) // NCORES          # 32 samples per core
S = 8                            # samples per block
NBLK = SPC // S                  # 4 blocks
W = S * Fd                       # 2048 free columns per block
HW_ = W // 2                     # 1024-wide psum half
P3 = 260                         # corr row pitch (guarded)
PX = 258                         # xn guarded pitch
SKEW = 3
LN_EPS = 1e-5
RSQRT_MAGIC = 0x5F3759DF


class _TileContext(tile.TileContext):
    """Walrus in this container caps sync-wait commands per CTRL-class
    instruction; spread the exit drain's waits across single-wait nops."""

    def _drain_and_barrier(self, tick_clock, wait_clock):
        drain_inst = self.nc.sync.drain()
        wait_clock.add_sem_waits(
            drain_inst.ins, ScopedClock({None: tick_clock.global_clock})
        )
        si = drain_inst.ins.sync_info
        waits = list(si.on_wait or []) if si else []
        if len(waits) > 1:
            si.on_wait = waits[:1]
            for w in waits[1:]:
                n = self.nc.sync.nop(nofuse=True).ins
                n.sync_info = mybir.SyncInfo(on_wait=[w], on_update=[])
        self.nc.all_engine_barrier()
        assert self.sems is not None
        popped = self.nc._tile_sem_poison_stack.pop()
        assert popped is self._sem_poison
        self.nc.clear_and_free_semaphores(list(self.sems.allocated().values()))
        self.nc.all_engine_barrier()


def _f32r(ap):
    return ap.bitcast(DT.float32r)


def _brep(ap, nrep):
    """Insert a stride-0 replication dim after the partition dim."""
    ap.ap.insert(1, [0, nrep])
    return ap


def _split_excess_waits(nc, max_waits=2):
    """Walrus in this container caps sync-wait commands per instruction.
    Move excess waits onto same-engine NoOps inserted just before."""
    for f in nc.m.functions:
        for bb in f.blocks:
            new_insts = []
            for inst in bb.instructions:
                si = inst.sync_info
                waits = list(si.on_wait) if si and si.on_wait else []
                if len(waits) > max_waits:
                    si.on_wait = waits[:max_waits]
                    rest = waits[max_waits:]
                    for i in range(0, len(rest), max_waits):
                        nop = mybir.InstEventSemaphore(
                            name=f"I-ws{nc.next_id()}", ins=[], outs=[])
                        nop.engine = inst.engine
                        nop.sync_info = mybir.SyncInfo(
                            on_wait=rest[i:i + max_waits], on_update=[])
                        nc.register_instruction(nop)
                        new_insts.append(nop)
                new_insts.append(inst)
            bb.instructions[:] = new_insts


def build_nc():
    nc = bass.Bass()

    # ---- DRAM I/O ----
    x_in = nc.dram_tensor("x", [C, SPC, Fd], DT.float32, kind="ExternalInput")
    out_d = nc.dram_tensor("out", [C, SPC, Fd], DT.float32,
                           kind="ExternalOutput")
    wqkv2_d = nc.dram_tensor("wqkv2", [128, 2, 3 * C], DT.float8e4,
                             kind="ExternalInput")
    corr2_d = nc.dram_tensor("corr2", [2, 3 * C], DT.bfloat16,
                             kind="ExternalInput")
    corrR_d = nc.dram_tensor("corrR", [2, 8 * 256], DT.bfloat16,
                             kind="ExternalInput")
    tapw_d = nc.dram_tensor("tapw", [128, 6, 2], DT.float32,
                            kind="ExternalInput")
    wproj_d = nc.dram_tensor("wproj", [128, 2, C], DT.bfloat16,
                             kind="ExternalInput")
    w1_d = nc.dram_tensor("w1", [128, 2, HID], DT.bfloat16,
                          kind="ExternalInput")
    w1cs_d = nc.dram_tensor("w1cs", [1, HID], DT.bfloat16,
                            kind="ExternalInput")
    w2_d = nc.dram_tensor("w2", [128, 4, C], DT.bfloat16,
                          kind="ExternalInput")
    b1v_d = nc.dram_tensor("b1v", [128, 4], DT.float32, kind="ExternalInput")
    b2v_d = nc.dram_tensor("b2v", [128, 2], DT.float32, kind="ExternalInput")
    temp_d = nc.dram_tensor("temp128", [128, 1], DT.float32,
                            kind="ExternalInput")
    onesb_d = nc.dram_tensor("onesb", [128, 128], DT.bfloat16,
                             kind="ExternalInput")
    bandh0_d = nc.dram_tensor("bandh0", [128, 128], DT.bfloat16,
                              kind="ExternalInput")
    bandh1_d = nc.dram_tensor("bandh1", [128, 128], DT.bfloat16,
                              kind="ExternalInput")
    rowind4_d = nc.dram_tensor("rowind4", [4, 128], DT.bfloat16,
                               kind="ExternalInput")
    ident_d = nc.dram_tensor("identb", [128, 128], DT.bfloat16,
                             kind="ExternalInput")


    with _TileContext(nc) as tc, contextlib.ExitStack() as ctx:
        cpool = ctx.enter_context(tc.tile_pool(name="consts", bufs=1))
        xpool = ctx.enter_context(tc.tile_pool(name="xp", bufs=2))
        xrpool = ctx.enter_context(tc.tile_pool(name="xrp", bufs=3))
        sqp = ctx.enter_context(tc.tile_pool(name="sqp", bufs=2))
        qkp = ctx.enter_context(tc.tile_pool(name="qkp", bufs=6))
        vvp = ctx.enter_context(tc.tile_pool(name="vvp", bufs=2))
        vpp = ctx.enter_context(tc.tile_pool(name="vpp", bufs=2))
        phap = ctx.enter_context(tc.tile_pool(name="phap", bufs=3))
        ostp = ctx.enter_context(tc.tile_pool(name="ostp", bufs=3))
        ohp = ctx.enter_context(tc.tile_pool(name="ohp", bufs=2))
        o1p = ctx.enter_context(tc.tile_pool(name="o1p", bufs=2))
        xhp = ctx.enter_context(tc.tile_pool(name="xhp", bufs=2))
        gelp = ctx.enter_context(tc.tile_pool(name="gelp", bufs=4))
        o2p = ctx.enter_context(tc.tile_pool(name="o2p", bufs=2))
        rows = ctx.enter_context(tc.tile_pool(name="rows", bufs=3))
        dense = ctx.enter_context(tc.tile_pool(name="dense", bufs=1))
        ps = ctx.enter_context(tc.tile_pool(name="ps", bufs=4, space="PSUM"))

        # ---- constants ----
        def cload(name, shape, dt, dram):
            t = cpool.tile(shape, dt, tag=name, name=name)
            nc.sync.dma_start(t[:], dram[:])
            return t

        wqkv_sb = cload("wqkv", [128, 2, 3 * C], DT.float8e4, wqkv2_d)
        corr2_sb = cload("corr2", [2, 3 * C], DT.bfloat16, corr2_d)
        tapw = cload("tapw", [128, 6, 2], DT.float32, tapw_d)
        wproj_sb = cload("wproj", [128, 2, C], DT.bfloat16, wproj_d)
        w1_sb = cload("w1", [128, 2, HID], DT.bfloat16, w1_d)
        w1cs_sb = cload("w1cs", [1, HID], DT.bfloat16, w1cs_d)
        w2_sb = cload("w2", [128, 4, C], DT.bfloat16, w2_d)
        b1v = cload("b1v", [128, 4], DT.float32, b1v_d)
        b2v = cload("b2v", [128, 2], DT.float32, b2v_d)
        temp128 = cload("temp", [128, 1], DT.float32, temp_d)
        onesb = cload("onesb", [128, 128], DT.bfloat16, onesb_d)
        bandh = [cload("bandh0", [128, 128], DT.bfloat16, bandh0_d),
                 cload("bandh1", [128, 128], DT.bfloat16, bandh1_d)]
        rowind4 = cload("rowind4", [4, 128], DT.bfloat16, rowind4_d)
        identb = cload("identb", [128, 128], DT.bfloat16, ident_d)

        # xn: single kt-interleaved fp8 tile (DoubleRow qkv rhs)
        xn8 = cpool.tile([128, 2, W], DT.float8e4, name="xn8", tag="xn8")
        # qpre: guarded staging for the depthwise conv (zero guard cols once)
        qpre_st = [cpool.tile([128, S, PX], DT.bfloat16,
                              name=f"qpre{i}", tag=f"qpre{i}")
                   for i in range(3)]
        for i in range(3):
            nc.vector.memset(qpre_st[i][:, :, 0:1], 0.0)
            nc.vector.memset(qpre_st[i][:, :, PX - 1:PX], 0.0)

        # ---- dense helpers (gpsimd newton chains, [*,128] tiles) ----
        def rsqrt_dense(x_ap, out_ap, iters=2, eng=None):
            eng = eng or nc.gpsimd
            shape = list(x_ap.shape)
            s1 = dense.tile(shape, DT.int32, tag="nw_i1", name="nw_i1")
            nc.vector.tensor_scalar(s1[:], x_ap.bitcast(DT.int32), 1, None,
                                    ALU.arith_shift_right)
            nc.vector.tensor_scalar(s1[:], s1[:], -1, None, ALU.bitwise_xor)
            nc.vector.tensor_scalar(s1[:], s1[:], RSQRT_MAGIC + 1, None,
                                    ALU.add)
            y = s1[:].bitcast(DT.float32)
            for it in range(iters):
                t = dense.tile(shape, DT.float32, tag="nw_t", name="nw_t")
                u = dense.tile(shape, DT.float32, tag="nw_u", name="nw_u")
                eng.tensor_mul(t[:], y, y)
                eng.tensor_scalar(t[:], t[:], -0.5, None, ALU.mult)
                eng.tensor_mul(u[:], t[:], x_ap)
                eng.tensor_scalar(u[:], u[:], 1.5, None, ALU.add)
                last = (it == iters - 1)
                ynew = out_ap if last else dense.tile(
                    shape, DT.float32, name="nw_y", tag="nw_y", bufs=2)
                yap = ynew if last else ynew[:]
                eng.tensor_mul(yap, u[:], y)
                y = yap

        def recip_dense(x_ap, out_ap, iters=2, eng=None):
            eng = eng or nc.gpsimd
            shape = list(x_ap.shape)
            s1 = dense.tile(shape, DT.int32, tag="nw_i1", name="nw_i1")
            nc.vector.tensor_scalar(s1[:], x_ap.bitcast(DT.int32), -1, None,
                                    ALU.bitwise_xor)
            nc.vector.tensor_scalar(s1[:], s1[:], 0x7EF127EA + 1, None,
                                    ALU.add)
            y = s1[:].bitcast(DT.float32)
            for it in range(iters):
                u = dense.tile(shape, DT.float32, tag="nw_t", name="nw_t")
                eng.tensor_mul(u[:], x_ap, y)
                v = dense.tile(shape, DT.float32, tag="nw_u", name="nw_u")
                eng.tensor_scalar(v[:], u[:], -1.0, None, ALU.mult)
                eng.tensor_scalar(v[:], v[:], 2.0, None, ALU.add)
                last = (it == iters - 1)
                ynew = out_ap if last else dense.tile(
                    shape, DT.float32, name="nw_y", tag="nw_y", bufs=2)
                yap = ynew if last else ynew[:]
                eng.tensor_mul(yap, v[:], y)
                y = yap

        st = {b: {} for b in range(NBLK)}

        # ---------------- stages ----------------
        def s0_load(blk):
            d = st[blk]
            d["xb"] = [sqp.tile([128, W], DT.bfloat16, name=f"xb_{kt}",
                                tag="xb") for kt in range(2)]
            for kt in range(2):
                for h2 in range(2):
                    xh32 = xpool.tile([128, S // 2, Fd], DT.float32,
                                      name="xh32", tag="x32")
                    nc.sync.dma_start(
                        xh32[:],
                        x_in[kt * 128:(kt + 1) * 128,
                             blk * S + h2 * 4:blk * S + (h2 + 1) * 4, :])
                    nc.vector.tensor_copy(
                        d["xb"][kt][:, h2 * HW_:(h2 + 1) * HW_],
                        xh32[:].rearrange("p s f -> p (s f)"))

        def _stats(blk, srcs, sqs, which, fr):
            """Partition-sum stats of srcs (and sqs) -> dense mu*rsig and
            rsig rows (bf16 [16,128]) stored as mr{which}/rr{which}."""
            d = st[blk]
            su_row = rows.tile([1, W], DT.bfloat16,
                               name=f"su_{which}", tag="rows")
            sq_row = rows.tile([1, W], DT.bfloat16,
                               name=f"sqr_{which}", tag="rows")
            for h2 in range(2):
                ps_su = ps.tile([1, HW_], DT.float32, tag="mm", name="ps_su")
                ps_sq = ps.tile([1, HW_], DT.float32, tag="mm", name="ps_sq")
                for ch in range(2):
                    sl = slice(h2 * HW_ + ch * 512, h2 * HW_ + (ch + 1) * 512)
                    psl = slice(ch * 512, (ch + 1) * 512)
                    for kt in range(2):
                        nc.tensor.matmul(
                            ps_su[0:1, psl], onesb[:, 0:1],
                            srcs[kt][:, sl],
                            start=(kt == 0), stop=(kt == 1),
                            skip_group_check=True)
                    for kt in range(2):
                        nc.tensor.matmul(
                            ps_sq[0:1, psl], onesb[:, 0:1],
                            sqs[(kt, h2)][:, psl],
                            start=(kt == 0), stop=(kt == 1),
                            skip_group_check=True)
                nc.scalar.activation(su_row[0:1, h2 * HW_:(h2 + 1) * HW_],
                                     ps_su[:], AF.Copy)
                nc.scalar.activation(sq_row[0:1, h2 * HW_:(h2 + 1) * HW_],
                                     ps_sq[:], AF.Copy)
            dsu = dense.tile([16, 128], DT.bfloat16, tag="dsu", name="dsu")
            dsq = dense.tile([16, 128], DT.bfloat16, tag="dsq", name="dsq")
            nc.sync.dma_start(
                dsu[:], su_row[:].rearrange("o (j c) -> o j c", c=128))
            nc.sync.dma_start(
                dsq[:], sq_row[:].rearrange("o (j c) -> o j c", c=128))
            mu = dense.tile([16, 128], DT.float32, tag="dmu", name="dmu")
            nc.vector.tensor_scalar(mu[:], dsu[:], 1.0 / C, None, ALU.mult)
            var = dense.tile([16, 128], DT.float32, tag="dvar", name="dvar")
            m2 = dense.tile([16, 128], DT.float32, tag="nw_t", name="dm2")
            nc.vector.tensor_mul(m2[:], mu[:], mu[:])
            nc.vector.tensor_scalar(var[:], dsq[:], 1.0 / C, LN_EPS,
                                    ALU.mult, ALU.add)
            nc.vector.tensor_sub(var[:], var[:], m2[:])
            rsd = dense.tile([16, 128], DT.float32, tag="drs", name="drs")
            rsqrt_dense(var[:], rsd[:], iters=1, eng=nc.vector)
            mr = dense.tile([16, 128], DT.bfloat16, tag="dmr", name="dmr")
            nc.vector.tensor_mul(mr[:], mu[:], rsd[:])
            rr = dense.tile([16, 128], DT.bfloat16, tag="drr", name="drr")
            nc.vector.tensor_copy(rr[:], rsd[:])
            d[f"mr{which}"] = mr
            d[f"rr{which}"] = rr

        def s1_ln1_stats(blk):
            d = st[blk]
            srcs = [d["xb"][kt][:] for kt in range(2)]
            sqs = {}
            for kt in range(2):
                for h2 in range(2):
                    x2 = sqp.tile([128, HW_], DT.bfloat16, name="x2",
                                  tag="sqh")
                    s_ = srcs[kt][:, h2 * HW_:(h2 + 1) * HW_]
                    nc.vector.scalar_tensor_tensor(
                        x2[:], s_, 1.0, s_, ALU.mult, ALU.mult)
                    sqs[(kt, h2)] = x2
            _stats(blk, srcs, sqs, 1, fr=False)

        def s2_ln1_apply(blk):
            d = st[blk]
            mr, rr = d.pop("mr1"), d.pop("rr1")
            rrow = rows.tile([1, W], DT.bfloat16, name="rrow", tag="rows")
            nc.sync.dma_start(
                rrow[:].rearrange("o (j c) -> o j c", c=128), rr[:])
            corrR = rows.tile([2, W], DT.bfloat16, name="corrR", tag="rows")
            nc.sync.dma_start(
                corrR[0:1, :].rearrange("o (j c) -> o j c", c=128), mr[:])
            nc.sync.dma_start(corrR[1:2, :], corrR_d[1:2, :])
            d["corrR"] = corrR
            for h2 in range(2):
                ps_r = ps.tile([128, HW_], DT.float32, tag="mm", name="ps_r")
                for ch in range(2):
                    nc.tensor.matmul(
                        ps_r[:, ch * 512:(ch + 1) * 512],
                        onesb[0:1, :],
                        rrow[0:1, h2 * HW_ + ch * 512:
                             h2 * HW_ + (ch + 1) * 512],
                        start=True, stop=True, skip_group_check=True)
                for kt in range(2):
                    nc.vector.tensor_mul(
                        xn8[:, kt, h2 * HW_:(h2 + 1) * HW_],
                        d["xb"][kt][:, h2 * HW_:(h2 + 1) * HW_],
                        ps_r[:])
            d["xn"] = xn8

        def qkv_m(blk, m, qc):
            d = st[blk]
            qp3 = qpre_st[m % 3][:]
            if m < 4:
                qt = qkp.tile([128, W], DT.bfloat16, name=f"qc{m}", tag="qk")
            else:
                qt = vvp.tile([128, W], DT.bfloat16, name=f"vc{m}", tag="vv")
            qc.append(qt)
            for h2 in range(2):
                ps_m = ps.tile([128, HW_], DT.float32, tag="mm", name="ps_m")
                for ch in range(2):
                    sl = slice(h2 * HW_ + ch * 512, h2 * HW_ + (ch + 1) * 512)
                    nc.tensor.matmul(
                        ps_m[:, ch * 512:(ch + 1) * 512],
                        wqkv_sb[:, :, m * 128:(m + 1) * 128],
                        d["xn"][:, :, sl],
                        start=True, stop=False,
                        perf_mode=mybir.MatmulPerfMode.DoubleRow,
                        skip_group_check=True)
                    nc.tensor.matmul(
                        ps_m[:, ch * 512:(ch + 1) * 512],
                        corr2_sb[:, m * 128:(m + 1) * 128],
                        d["corrR"][:, sl],
                        start=False, stop=True, skip_group_check=True)
                nc.vector.tensor_scalar(
                    qp3[:, h2 * 4:(h2 + 1) * 4, 1:1 + Fd],
                    ps_m[:].rearrange("p (s f) -> p s f", f=Fd),
                    1.0 / 16.0, None, ALU.mult)
            # depthwise conv3 along f (middle tap folded into Wqkv):
            # qc = qpre + r0*shift(-1) + r2*shift(+1), in place (bf16 DVE)
            vm1 = qp3[:, :, 0:Fd]
            v00 = qp3[:, :, 1:1 + Fd]
            vp1 = qp3[:, :, 2:2 + Fd]
            qf = qc[m][:].rearrange("p (s f) -> p s f", f=Fd)
            nc.vector.scalar_tensor_tensor(qf, vm1, tapw[:, m, 0:1], v00,
                                           ALU.mult, ALU.add)
            nc.vector.scalar_tensor_tensor(qf, vp1, tapw[:, m, 1:2], qf,
                                           ALU.mult, ALU.add)

        def qkv_tail(blk, qc):
            d = st[blk]
            d.pop("corrR")
            d["qc"] = qc[:4]
            # v transpose (frees v tiles fast)
            vp = [vpp.tile([128, S, H, Dh + 1], DT.bfloat16,
                           name=f"vp{b2}", tag=f"vp{b2}") for b2 in range(2)]
            for b2 in range(2):
                nc.vector.memset(vp[b2][:, :, :, Dh:Dh + 1], 1.0)
            for ti in range(2):
                vt = qc[4 + ti]
                for b2 in range(2):
                    ps_tp = ps.tile([128, S * 128], DT.bfloat16, tag="mm",
                                    name="ps_tp")
                    for s in range(S):
                        nc.tensor.transpose(
                            ps_tp[:, s * 128:(s + 1) * 128],
                            vt[:, s * Fd + b2 * 128:s * Fd + b2 * 128 + 128],
                            identb[:])
                    nc.vector.tensor_copy(
                        vp[b2][:, :, 4 * ti:4 * ti + 4, 0:Dh],
                        ps_tp[:].rearrange("p (s hb d) -> p s hb d",
                                           s=S, hb=4))
            d["vp"] = vp

        def s3_qkv(blk):
            qc = []
            for m in range(6):
                qkv_m(blk, m, qc)
            qkv_tail(blk, qc)

        def s4_l2sumsq(blk):
            d = st[blk]
            d["dnq"] = {}
            for vi, base in (("q", 0), ("k", 2)):
                d_n = dense.tile([128, 128], DT.bfloat16, tag="dn", name="dn")
                for h2 in range(2):
                    ps_n = ps.tile([128, HW_], DT.float32, tag="mm",
                                   name="ps_n")
                    for ti in range(2):
                        sq = sqp.tile([128, HW_], DT.bfloat16, name="l2sq",
                                      tag="sqh")
                        qs = d["qc"][base + ti][:, h2 * HW_:(h2 + 1) * HW_]
                        nc.scalar.activation(sq[:], qs, AF.Square)
                        for ch in range(2):
                            nc.tensor.matmul(
                                ps_n[:, ch * 512:(ch + 1) * 512],
                                bandh[ti][:], sq[:, ch * 512:(ch + 1) * 512],
                                start=(ti == 0), stop=(ti == 1),
                                skip_group_check=True)
                    nsb = rows.tile([8, HW_], DT.bfloat16, name="nsb",
                                    tag="rows")
                    nc.any.tensor_copy(nsb[:], ps_n[0:8, :])
                    nc.sync.dma_start(
                        d_n[h2 * 64:(h2 + 1) * 64, :],
                        nsb[:].rearrange("h (j c) -> h j c", c=128))
                dnf = dense.tile([128, 128], DT.float32, tag="dnf",
                                 name="dnf")
                nc.vector.tensor_copy(dnf[:], d_n[:])
                r_n = dense.tile([128, 128], DT.float32, tag="dr", name="dr")
                rsqrt_dense(dnf[:], r_n[:], iters=1, eng=nc.vector)
                r16 = dense.tile([128, 128], DT.bfloat16, tag="dr16",
                                 name="dr16")
                if vi == "k":
                    nc.vector.tensor_scalar(r16[:], r_n[:], temp128[:, 0:1],
                                            None, ALU.mult)
                else:
                    nc.vector.tensor_copy(r16[:], r_n[:])
                d["dnq"][vi] = r16

        def s5_l2apply(blk):
            d = st[blk]
            for vi, base in (("q", 0), ("k", 2)):
                r16 = d["dnq"].pop(vi)
                for ti in range(2):
                    r4 = rows.tile([4, W], DT.bfloat16, name="r4", tag="rows")
                    for h2 in range(2):
                        nc.sync.dma_start(
                            r4[:, h2 * HW_:(h2 + 1) * HW_].rearrange(
                                "b (j c) -> b j c", c=128),
                            r16[h2 * 64 + ti * 32:h2 * 64 + ti * 32 + 32, :])
                    for h2 in range(2):
                        ps_b = ps.tile([128, HW_], DT.float32, tag="mm",
                                       name="ps_b")
                        for ch in range(2):
                            sl = slice(h2 * HW_ + ch * 512,
                                       h2 * HW_ + (ch + 1) * 512)
                            nc.tensor.matmul(
                                ps_b[:, ch * 512:(ch + 1) * 512],
                                rowind4[:], r4[:, sl],
                                start=True, stop=True, skip_group_check=True)
                        qs = d["qc"][base + ti][:, h2 * HW_:(h2 + 1) * HW_]
                        nc.vector.tensor_mul(qs, qs, ps_b[:])
            d.pop("dnq")

        def attn_h(blk, h, oh16, d_rs):
            d = st[blk]
            ti, band = h // 4, (h % 4) * 32
            phat = []
            for jt in range(2):
                pj = phap.tile([128, W], DT.bfloat16, tag="phat",
                               name="phat")
                for h2 in range(2):
                    ps_S = ps.tile([128, HW_], DT.float32, tag="mm",
                                   name="ps_S")
                    for si in range(4):
                        s = h2 * 4 + si
                        nc.tensor.matmul(
                            ps_S[:, si * Fd:(si + 1) * Fd],
                            d["qc"][2 + ti][
                                band:band + 32,
                                s * Fd + jt * 128:s * Fd + jt * 128 + 128],
                            d["qc"][ti][band:band + 32,
                                        s * Fd:(s + 1) * Fd],
                            start=True, stop=True, skip_group_check=True,
                            tile_position=(band, 0))
                    nc.scalar.activation(
                        pj[:, h2 * HW_:(h2 + 1) * HW_], ps_S[:], AF.Exp)
                phat.append(pj)
            ostg = ostp.tile([Dh + 1, W], DT.bfloat16, tag="ostg",
                             name="ostg")
            for h2 in range(2):
                ps_O = ps.tile([Dh + 1, HW_], DT.float32, tag="mm",
                               name="ps_O")
                for si in range(4):
                    s = h2 * 4 + si
                    for ktj in range(2):
                        nc.tensor.matmul(
                            ps_O[:, si * Fd:(si + 1) * Fd],
                            d["vp"][ktj][:, s, h, :],
                            phat[ktj][:, s * Fd:(s + 1) * Fd],
                            start=(ktj == 0), stop=(ktj == 1),
                            skip_group_check=True)
                nc.any.tensor_copy(ostg[:, h2 * HW_:(h2 + 1) * HW_],
                                   ps_O[:])
            nc.gpsimd.dma_start(oh16[ti][band:band + 32, :],
                                ostg[0:Dh, :])
            nc.sync.dma_start(
                d_rs[ti][(h % 4) * 16:(h % 4 + 1) * 16, :],
                ostg[Dh:Dh + 1, :].rearrange("o (j c) -> o j c", c=128))

        def attn_head_setup(blk):
            d = st[blk]
            oh16 = [ohp.tile([128, W], DT.bfloat16, name=f"oh{ti}", tag="oh")
                    for ti in range(2)]
            d_rs = [dense.tile([64, 128], DT.bfloat16, tag=f"dnr{ti}",
                               name=f"dnr{ti}") for ti in range(2)]
            d["oh16"] = oh16
            return oh16, d_rs

        def attn_tail_ti(blk, ti, d_rs):
            d = st[blk]
            drf = dense.tile([64, 128], DT.float32, tag="drf", name="drf",
                             bufs=2)
            nc.vector.tensor_copy(drf[:], d_rs[ti][:])
            d_ri = dense.tile([64, 128], DT.bfloat16, tag=f"dri{ti}",
                              name=f"dri{ti}")
            recip_dense(drf[:], d_ri[:], eng=nc.vector)
            d[f"d_ri{ti}"] = d_ri

        def s6_attn(blk):
            oh16, d_rs = attn_head_setup(blk)
            for h in range(H):
                attn_h(blk, h, oh16, d_rs)
                if h == 3:
                    attn_tail_ti(blk, 0, d_rs)
            attn_tail_ti(blk, 1, d_rs)

        def s7_denom(blk, ti):
            d = st[blk]
            d_ri = d.pop(f"d_ri{ti}")
            r4 = rows.tile([4, W], DT.bfloat16, name="rinv", tag="rows")
            nc.sync.dma_start(
                r4[:].rearrange("b (j c) -> b j c", c=128),
                d_ri[:])
            for h2 in range(2):
                ps_b = ps.tile([128, HW_], DT.float32, tag="mm",
                               name="ps_b2")
                for ch in range(2):
                    sl = slice(h2 * HW_ + ch * 512,
                               h2 * HW_ + (ch + 1) * 512)
                    nc.tensor.matmul(
                        ps_b[:, ch * 512:(ch + 1) * 512],
                        rowind4[:], r4[:, sl],
                        start=True, stop=True, skip_group_check=True)
                ohs = d["oh16"][ti][:, h2 * HW_:(h2 + 1) * HW_]
                nc.vector.tensor_mul(ohs, ohs, ps_b[:])

        def s7_proj(blk):
            d = st[blk]
            d["xr"] = {}
            for m2 in range(2):
                for h2 in range(2):
                    xr32 = xrpool.tile([128, S // 2, Fd], DT.float32,
                                       name="xr32", tag="xr32")
                    nc.sync.dma_start(
                        xr32[:],
                        x_in[m2 * 128:(m2 + 1) * 128,
                             blk * S + h2 * 4:blk * S + (h2 + 1) * 4, :])
                    d["xr"][(m2, h2)] = xr32
            for ti in range(2):
                if f"d_ri{ti}" in d:
                    s7_denom(blk, ti)
            o1 = [o1p.tile([128, W], DT.bfloat16, name=f"o1_{m}", tag="o1")
                  for m in range(2)]
            for m2 in range(2):
                for h2 in range(2):
                    xr32 = xrpool.tile([128, S // 2, Fd], DT.float32,
                                       name="xr32", tag="xr32")
                    nc.sync.dma_start(
                        xr32[:],
                        x_in[m2 * 128:(m2 + 1) * 128,
                             blk * S + h2 * 4:blk * S + (h2 + 1) * 4, :])
                    xf = xr32[:].rearrange("p s f -> p (s f)")
                    ps_y = ps.tile([128, HW_], DT.float32, tag="mm",
                                   name="ps_y")
                    for ch in range(2):
                        sl = slice(h2 * HW_ + ch * 512,
                                   h2 * HW_ + (ch + 1) * 512)
                        for kt in range(2):
                            nc.tensor.matmul(
                                ps_y[:, ch * 512:(ch + 1) * 512],
                                wproj_sb[:, kt, m2 * 128:(m2 + 1) * 128],
                                d["oh16"][kt][:, sl],
                                start=(kt == 0), stop=(kt == 1),
                                skip_group_check=True)
                    nc.vector.scalar_tensor_tensor(
                        o1[m2][:, h2 * HW_:(h2 + 1) * HW_],
                        ps_y[:], 1.0, xf[:], ALU.mult, ALU.add)
            d["o1"] = o1
            d.pop("oh16")
            d.pop("qc")
            d.pop("vp")
            d.pop("xn", None)
            d.pop("xb")

        def s8_ln2_stats(blk):
            d = st[blk]
            srcs = [d["o1"][kt][:] for kt in range(2)]
            sqs = {}
            for kt in range(2):
                for h2 in range(2):
                    x2 = sqp.tile([128, HW_], DT.bfloat16, name="o1sq",
                                  tag="sqh")
                    s_ = srcs[kt][:, h2 * HW_:(h2 + 1) * HW_]
                    nc.scalar.activation(x2[:], s_, AF.Square)
                    sqs[(kt, h2)] = x2
            _stats(blk, srcs, sqs, 2, fr=False)

        def mlp_head(blk):
            d = st[blk]
            mr, rr = d.pop("mr2"), d.pop("rr2")
            rrow = rows.tile([1, W], DT.bfloat16, name="rrow2", tag="rows")
            nc.sync.dma_start(
                rrow[:].rearrange("o (j c) -> o j c", c=128), rr[:])
            mrow = rows.tile([1, W], DT.bfloat16, name="mrow2", tag="rows")
            nc.sync.dma_start(
                mrow[:].rearrange("o (j c) -> o j c", c=128), mr[:])
            xh = [xhp.tile([128, W], DT.bfloat16, name=f"xh{kt}", tag="xh")
                  for kt in range(2)]
            for h2 in range(2):
                ps_r = ps.tile([128, HW_], DT.float32, tag="mm", name="ps_r2")
                for ch in range(2):
                    nc.tensor.matmul(
                        ps_r[:, ch * 512:(ch + 1) * 512],
                        onesb[0:1, :],
                        rrow[0:1, h2 * HW_ + ch * 512:
                             h2 * HW_ + (ch + 1) * 512],
                        start=True, stop=True, skip_group_check=True)
                for kt in range(2):
                    nc.vector.tensor_mul(
                        xh[kt][:, h2 * HW_:(h2 + 1) * HW_],
                        d["o1"][kt][:, h2 * HW_:(h2 + 1) * HW_],
                        ps_r[:])
            g16 = [gelp.tile([128, W], DT.bfloat16, name=f"gel{m}", tag="gel")
                   for m in range(4)]
            d["xh"] = xh
            d["mrow"] = mrow
            d["g16"] = g16

        def mlp_w1(blk, mh):
            d = st[blk]
            xh, mrow, g16 = d["xh"], d["mrow"], d["g16"]
            for h2 in range(2):
                ps_h = ps.tile([128, HW_], DT.float32, tag="mm",
                               name="ps_h")
                for ch in range(2):
                    sl = slice(h2 * HW_ + ch * 512,
                               h2 * HW_ + (ch + 1) * 512)
                    for kt in range(2):
                        nc.tensor.matmul(
                            ps_h[:, ch * 512:(ch + 1) * 512],
                            w1_sb[:, kt, mh * 128:(mh + 1) * 128],
                            xh[kt][:, sl],
                            start=(kt == 0), stop=False,
                            skip_group_check=True)
                    nc.tensor.matmul(
                        ps_h[:, ch * 512:(ch + 1) * 512],
                        w1cs_sb[0:1, mh * 128:(mh + 1) * 128],
                        mrow[0:1, sl],
                        start=False, stop=True, skip_group_check=True)
                nc.scalar.activation(
                    g16[mh][:, h2 * HW_:(h2 + 1) * HW_], ps_h[:],
                    AF.Gelu, bias=b1v[:, mh:mh + 1], scale=1.0)

        def mlp_w2(blk, m2, h2):
            d = st[blk]
            g16 = d["g16"]
            ps_y = ps.tile([128, HW_], DT.float32, tag="mm", name="ps_y2")
            for ch in range(2):
                for kt in range(4):
                    nc.tensor.matmul(
                        ps_y[:, ch * 512:(ch + 1) * 512],
                        w2_sb[:, kt, m2 * 128:(m2 + 1) * 128],
                        g16[kt][:, h2 * HW_ + ch * 512:
                                h2 * HW_ + (ch + 1) * 512],
                        start=(kt == 0), stop=(kt == 3),
                        skip_group_check=True)
            o2 = o2p.tile([128, HW_], DT.float32, tag="o2", name="o2t")
            nc.vector.scalar_tensor_tensor(
                o2[:], ps_y[:], b2v[:, m2:m2 + 1],
                d["o1"][m2][:, h2 * HW_:(h2 + 1) * HW_],
                ALU.add, ALU.add)
            nc.sync.dma_start(
                out_d[m2 * 128:(m2 + 1) * 128,
                      blk * S + h2 * 4:blk * S + (h2 + 1) * 4, :],
                o2[:].rearrange("p (s f) -> p s f", f=Fd))

        def mlp_tail(blk):
            d = st[blk]
            d.pop("xh")
            d.pop("mrow")
            d.pop("g16")
            d.pop("o1")

        def s9_mlp(blk):
            mlp_head(blk)
            for mh in range(4):
                mlp_w1(blk, mh)
            for m2 in range(2):
                for h2 in range(2):
                    mlp_w2(blk, m2, h2)
            mlp_tail(blk)

        def merged_step(bm, ba, bq):
            """Interleave mlp(bm), attention(ba), qkv(bq); any may be None."""
            if ba is not None:
                oh16, d_rs = attn_head_setup(ba)
            if bm is not None:
                mlp_head(bm)
            qc = []
            plan = []
            for i in range(8):
                if ba is not None:
                    plan.append(("h", i))
                if bm is not None and i < 4:
                    plan.append(("w1", i))
                if bm is not None and 4 <= i < 8:
                    plan.append(("w2", i - 4))
                if bq is not None and 1 <= i < 7:
                    plan.append(("m", i - 1))
            for kind, i in plan:
                if kind == "h":
                    attn_h(ba, i, oh16, d_rs)
                    if i == 3:
                        attn_tail_ti(ba, 0, d_rs)
                    elif i == 7:
                        attn_tail_ti(ba, 1, d_rs)
                elif kind == "w1":
                    mlp_w1(bm, i)
                elif kind == "w2":
                    mlp_w2(bm, i // 2, i % 2)
                else:
                    qkv_m(bq, i, qc)
            if bq is not None:
                qkv_tail(bq, qc)
            if bm is not None:
                mlp_tail(bm)

        stages = [s0_load, s1_ln1_stats, s2_ln1_apply, s3_qkv, s4_l2sumsq,
                  s5_l2apply, s6_attn, s7_proj, s8_ln2_stats, s9_mlp]


        # skew-3 wavefront: later stages (lower block index) first.
        # s9(b-1), s6(b), s3(b+1) land on the same step; emit them
        # interleaved so the shared psum ring rotates through all three.
        nst = len(stages)
        for t in range(nst + SKEW * (NBLK - 1)):
            todo = [(b, t - SKEW * b) for b in range(NBLK)
                    if 0 <= t - SKEW * b < nst]
            jmap = {j: b for (b, j) in todo}
            skip = set()
            if 6 in jmap:
                bm = jmap.get(9)
                ba = jmap[6]
                bq = jmap.get(3)
                for j in (9, 6, 3):
                    if j in jmap:
                        skip.add((jmap[j], j))
            order = sorted(todo, key=lambda bj: (0 if bj[1] == 4 else 1,
                                                 bj[0]))
            for b, j in order:
                if (b, j) in skip:
                    if j == 6:
                        merged_step(jmap.get(9), b, jmap.get(3))
                    continue
                stages[j](b)

    _split_excess_waits(nc, max_waits=1)
    return nc


def _host_prep(inputs):
    Wqkv = np.asarray(inputs["Wqkv"], np.float64)        # (C, 3C)
    g1 = np.asarray(inputs["norm1_g"], np.float64)
    b1n = np.asarray(inputs["norm1_b"], np.float64)
    g2 = np.asarray(inputs["norm2_g"], np.float64)
    b2n = np.asarray(inputs["norm2_b"], np.float64)
    dw_w = np.asarray(inputs["dw_w"], np.float64)
    taps = dw_w[:, 0, :]                                 # (3C, 3)

    Wq = Wqkv * g1[:, None]                              # g1 folded
    # fold the middle conv tap into the weights; conv uses tap ratios
    w1t = taps[:, 1].copy()
    w1t = np.where(np.abs(w1t) < 1e-30, 1e-30, w1t)
    Wqf = Wq * w1t[None, :] * 16.0
    import ml_dtypes as _mld
    wqkv2 = np.ascontiguousarray(
        Wqf.reshape(2, 128, 3 * C).transpose(1, 0, 2)).astype(
            _mld.float8_e4m3)

    colsum = wqkv2.astype(np.float64).transpose(1, 0, 2).reshape(
        C, 3 * C).sum(axis=0)                            # (3C,) post-quant
    bq = (b1n @ Wqkv) * w1t * 16.0                       # (3C,)
    corr2 = np.stack([-colsum, bq]).astype(BF16)         # (2, 3C)
    corrR = np.zeros((2, S * Fd), np.float32)
    corrR[1] = 1.0                                       # static ones row
    ratios = np.stack([taps[:, 0] / w1t, taps[:, 2] / w1t], axis=1)
    tapw = np.ascontiguousarray(
        ratios.reshape(6, 128, 2).transpose(1, 0, 2)).astype(np.float32)

    def kt_major(w, nkt):
        K, N = w.shape
        return np.ascontiguousarray(
            w.reshape(nkt, 128, N).transpose(1, 0, 2)).astype(BF16)

    wproj = kt_major(np.asarray(inputs["Wproj"], np.float64), 2)
    W1 = np.asarray(inputs["W1"], np.float64)
    W1g = W1 * g2[:, None]
    w1 = kt_major(W1g, 2)
    w1cs = (-W1g.sum(axis=0)).reshape(1, HID).astype(BF16)
    w2 = kt_major(np.asarray(inputs["W2"], np.float64), 4)

    b1p = np.asarray(inputs["b1"], np.float64) + b2n @ W1
    b1v = np.ascontiguousarray(b1p.reshape(4, 128).T).astype(np.float32)
    b2v = np.ascontiguousarray(
        np.asarray(inputs["b2"], np.float32).reshape(2, 128).T)

    temp = np.asarray(inputs["temperature"], np.float32).reshape(H)
    # l2 dense rows are h2-major: row = h2*64 + head*8 + j
    temp128 = np.array([temp[(r % 64) // 8] for r in range(128)],
                       np.float32).reshape(128, 1)

    bandh0 = np.zeros((128, 128), np.float32)
    bandh1 = np.zeros((128, 128), np.float32)
    for dd in range(128):
        for m in range(128):
            if m % 8 == dd // 32:
                bandh0[dd, m] = 1.0
            if m % 8 == 4 + dd // 32:
                bandh1[dd, m] = 1.0
    rowind4 = np.zeros((4, 128), np.float32)
    for m in range(128):
        rowind4[m // 32, m] = 1.0

    return dict(
        wqkv2=wqkv2, corr2=corr2, corrR=corrR.astype(BF16), tapw=tapw,
        wproj=wproj, w1=w1, w1cs=w1cs, w2=w2,
        b1v=b1v, b2v=b2v, temp128=temp128,
        onesb=np.ones((128, 128), BF16),
        bandh0=bandh0.astype(BF16),
        bandh1=bandh1.astype(BF16),
        rowind4=rowind4.astype(BF16),
        identb=np.eye(128).astype(BF16),
    )


_NC_CACHE = {}


def get_nc():
    if "nc" not in _NC_CACHE:
        _NC_CACHE["nc"] = build_nc()
    return _NC_CACHE["nc"]


def make_in_maps(inputs):
    consts = _host_prep(inputs)
    x = np.asarray(inputs["x"], np.float32)  # (B, C, T, Fd)
    in_maps = []
    for core in range(NCORES):
        b, t0 = core // 2, (core % 2) * SPC
        m = dict(consts)
        m["x"] = np.ascontiguousarray(x[b, :, t0:t0 + SPC, :])
        in_maps.append(m)
    return in_maps


def assemble_out(results):
    out = np.zeros((B, C, T, Fd), np.float32)
    for core in range(NCORES):
        b, t0 = core // 2, (core % 2) * SPC
        out[b, :, t0:t0 + SPC, :] = results[core]["out"]
    return out


def kernel(**inputs):
    nc = get_nc()
    in_maps = make_in_maps(inputs)
    res = run_bass_kernel_spmd(nc, in_maps, core_ids=list(range(NCORES)))
    return assemble_out(res.results)
